# revision 11
# baseline (speedup 1.0000x reference)
"""Trainium2 Bass kernel for nn_CG_model (GNN message passing).

Edge parallelism across 8 NeuronCores (per sharding hint): each core holds the
full node set, processes E/8 edges, and produces a partial N-sized scatter
accumulator; the host sums the 8 partials (the all-reduce) and applies the
final per-node 1/T scale.

Device-side structure (per core):
 - Node phase: MonotonicMLP U-net; the four (S,V) finite-difference variants
   ride one shared matmul with per-partition ACT bias shifts. Produces
   T, 1/T, 1/(C T), 1/C, P/d^2 and writes a 256B-row node table to DRAM
   (partition-major remapped node ids so table writes are contiguous).
 - Edge phase in superchunks of 2048 edges: dma_gather (HW token gather,
   1024 idx/call) pulls i- and j-node channels edge-major; geometry and the
   whole payload combine run on DVE edge-major planes; the A/B/C/W MLPs run
   feature-major on the PE with T+eps / r+-eps variants as ACT bias shifts;
   l3 results are written at 32-row PSUM bases, PE-transposed back to
   edge-major; dma_scatter_add pushes 8-channel payloads (u3, si / -u3, sj)
   into a 256B-strided accumulator.
 - dma_scatter_add drops colliding updates, so the host pre-groups edges into
   1024-token windows with all-distinct i and all-distinct j; window padding
   targets unique trash rows past the node range. Calls serialize on the
   accumulator WAW chain.
"""

import numpy as np

import concourse.bass as bass
import concourse.bacc as bacc
import concourse.tile as tile
import concourse.mybir as mybir
from concourse import bass_utils
from concourse.masks import make_identity

F32 = mybir.dt.float32
I16 = mybir.dt.int16
AF = mybir.ActivationFunctionType
TT = mybir.AluOpType

# problem constants (hardcoded per harness contract)
N, E, D, H, DT, HID = 20000, 640000, 3, 1.0, 0.01, 64
NCORES = 8
EPC = E // NCORES
P = 128
NB = 157                     # node blocks; NP = 128*157 = 20096 >= N
NP = P * NB
NI = 1024                    # tokens per gather/scatter call
ET = 16                      # edge tiles per superchunk
SC = P * ET                  # 2048 edges per superchunk
HSC = SC // 2                # 1024 (half superchunk = one gather call)
NR = NP + NI                 # accumulator rows incl. trash window
EPS_T = 1e-3
EPS_U = 0.01
CW_COEF = float(np.sqrt(2.0) / np.sqrt(DT))

_PROG_CACHE = {}


def _remap(n):
    return (n % P) * NB + n // P


def _wrap16(idx):
    w = np.asarray(idx).reshape(NI // 16, 16).T.astype(np.int16)
    return np.tile(w, (8, 1))


def _group_edges(i, j, cap=NI):
    """Partition edge ids into groups of <=cap with all-distinct i and j.
    Each round takes the first-occurrence set for both endpoints (any subset
    of it is still distinct) and emits only full cap-sized groups, recycling
    the remainder so almost every group is full."""
    rem = np.arange(len(i))
    groups = []
    while len(rem):
        fi = np.zeros(len(rem), bool)
        fj = np.zeros(len(rem), bool)
        _, ui = np.unique(i[rem], return_index=True)
        _, uj = np.unique(j[rem], return_index=True)
        fi[ui] = True
        fj[uj] = True
        take = np.where(fi & fj)[0]
        nfull = len(take) // cap
        if nfull >= 1 and len(take) > nfull * cap and len(rem) > len(take):
            take = take[:nfull * cap]          # recycle the partial group
        for o in range(0, len(take), cap):
            groups.append(rem[take[o:o + cap]])
        keep = np.ones(len(rem), bool)
        keep[take] = False
        rem = rem[keep]
    return groups


def _build_program(nsc, b3):
    nc = bacc.Bacc("TRN2", target_bir_lowering=False, debug=False,
                   num_devices=NCORES)
    ETOT = nsc * ET

    t_in = lambda nm, shp, dt=F32: nc.dram_tensor(nm, shp, dt, kind="ExternalInput").ap()
    S_nm = t_in("S_nm", [P, NB])
    d_nm = t_in("d_nm", [P, NB])
    v_nm = t_in("v_nm", [P, NB, 3])
    vv_half = t_in("vv_half", [P, NB])
    epack = t_in("epack", [P, ETOT, 14])
    gi_idx = t_in("gi_idx", [nsc, P, 2 * (NI // 16)], I16)
    gj_idx = t_in("gj_idx", [nsc, P, 2 * (NI // 16)], I16)
    si_idx = t_in("si_idx", [nsc, P, 2 * (NI // 16)], I16)
    sj_idx = t_in("sj_idx", [nsc, P, 2 * (NI // 16)], I16)
    U1 = t_in("U1", [2, HID])
    U2 = t_in("U2", [P, P])
    U3 = t_in("U3", [P, 2])
    Ub1v = t_in("Ub1v", [HID, 4])
    Ub2 = t_in("Ub2", [P, 1])
    L1_ABi = t_in("L1_ABi", [4, P])
    L1_ABj = t_in("L1_ABj", [4, P])
    L1_CWi = t_in("L1_CWi", [4, P])
    L1_CWj = t_in("L1_CWj", [4, P])
    L2_AB = t_in("L2_AB", [P, P])
    L2_CW = t_in("L2_CW", [P, P])
    L3_AB = t_in("L3_AB", [P, 32])
    L3_CW = t_in("L3_CW", [P, 32])
    B1_AB = t_in("B1_AB", [P, 2])
    B1_CW = t_in("B1_CW", [P, 2])
    B2_AB = t_in("B2_AB", [P, 1])
    B2_CW = t_in("B2_CW", [P, 1])

    acc = nc.dram_tensor("acc", [NR, 64], F32, kind="Internal").ap()
    acc_out = nc.dram_tensor("acc_out", [NP, 8], F32, kind="ExternalOutput").ap()
    invT_out = nc.dram_tensor("invT_out", [P, NB], F32, kind="ExternalOutput").ap()
    eout = nc.dram_tensor("eout", [P, NB], F32, kind="ExternalOutput").ap()

    node_tab = nc.dram_tensor("node_tab", [NP * 64], F32, kind="Internal").ap()
    sv_stage = nc.dram_tensor("sv_stage", [2, NP], F32, kind="Internal").ap()
    u_stage = nc.dram_tensor("u_stage", [4, NP], F32, kind="Internal").ap()

    with tile.TileContext(nc) as tc:
        with tc.tile_pool(name="const", bufs=1) as cpool:
            ident = cpool.tile([P, P], F32)
            make_identity(nc, ident[:])

            def ldc(ap_in, shape, dt=F32):
                t = cpool.tile(shape, dt, tag=ap_in.tensor.name, name=ap_in.tensor.name)
                nc.sync.dma_start(out=t[:], in_=ap_in)
                return t

            cU1 = ldc(U1, [2, HID]); cU2 = ldc(U2, [P, P]); cU3 = ldc(U3, [P, 2])
            cUb1 = ldc(Ub1v, [HID, 4]); cUb2 = ldc(Ub2, [P, 1])
            cL1 = {"ABi": ldc(L1_ABi, [4, P]), "ABj": ldc(L1_ABj, [4, P]),
                   "CWi": ldc(L1_CWi, [4, P]), "CWj": ldc(L1_CWj, [4, P])}
            cL2 = {"AB": ldc(L2_AB, [P, P]), "CW": ldc(L2_CW, [P, P])}
            cL3 = {"AB": ldc(L3_AB, [P, 32]), "CW": ldc(L3_CW, [P, 32])}
            cB1 = {"AB": ldc(B1_AB, [P, 2]), "CW": ldc(B1_CW, [P, 2])}
            cB2 = {"AB": ldc(B2_AB, [P, 1]), "CW": ldc(B2_CW, [P, 1])}

            # ================= node phase =================
            with tc.tile_pool(name="node", bufs=1) as npool, \
                 tc.tile_pool(name="npsA", bufs=1, space="PSUM") as npsA, \
                 tc.tile_pool(name="npsB", bufs=1, space="PSUM") as npsB:
                nS = npool.tile([P, NB], F32)
                nD = npool.tile([P, NB], F32)
                nV = npool.tile([P, NB], F32)
                nc.sync.dma_start(out=nS[:], in_=S_nm[:])
                nc.sync.dma_start(out=nD[:], in_=d_nm[:])
                nc.vector.reciprocal(out=nV[:], in_=nD[:])
                nc.sync.dma_start(out=sv_stage[0, :].rearrange("(p b) -> p b", p=P), in_=nS[:])
                nc.sync.dma_start(out=sv_stage[1, :].rearrange("(p b) -> p b", p=P), in_=nV[:])

                CH = 2048
                off = 0
                while off < NP:
                    cw = min(CH, NP - off)
                    sv = npool.tile([2, CH], F32, tag="sv", name="sv")
                    nc.sync.dma_start(out=sv[:, :cw], in_=sv_stage[:, off:off + cw])
                    ps1 = npsA.tile([P, CH], F32, tag="ups", name="ups")
                    o = 0
                    while o < cw:
                        n = min(512, cw - o)
                        nc.tensor.matmul(out=ps1[:HID, o:o + n], lhsT=cU1[:],
                                         rhs=sv[:, o:o + n], start=True, stop=True)
                        o += n
                    h1a = npool.tile([P, CH], F32, tag="h1a", name="h1a")
                    h1b = npool.tile([P, CH], F32, tag="h1b", name="h1b")
                    for dst, bcol in ((h1a[:HID, :cw], 0), (h1a[HID:, :cw], 1),
                                      (h1b[:HID, :cw], 2), (h1b[HID:, :cw], 3)):
                        nc.scalar.activation(out=dst, in_=ps1[:HID, :cw], func=AF.Exp, bias=cUb1[:, bcol:bcol + 1])
                        nc.vector.tensor_scalar(out=dst, in0=dst, scalar1=1.0, scalar2=None, op0=TT.add)
                        nc.scalar.activation(out=dst, in_=dst, func=AF.Ln)
                    for hbuf, r0 in ((h1a, 0), (h1b, 2)):
                        ps2 = npsA.tile([P, CH], F32, tag="ups", name="ups")
                        o = 0
                        while o < cw:
                            n = min(512, cw - o)
                            nc.tensor.matmul(out=ps2[:, o:o + n], lhsT=cU2[:],
                                             rhs=hbuf[:, o:o + n], start=True, stop=True)
                            o += n
                        h2 = npool.tile([P, CH], F32, tag="uh2", name="uh2")
                        nc.scalar.activation(out=h2[:, :cw], in_=ps2[:, :cw], func=AF.Exp, bias=cUb2[:, 0:1])
                        nc.vector.tensor_scalar(out=h2[:, :cw], in0=h2[:, :cw], scalar1=1.0, scalar2=None, op0=TT.add)
                        nc.scalar.activation(out=h2[:, :cw], in_=h2[:, :cw], func=AF.Ln)
                        ps3 = npsB.tile([2, CH], F32, tag="ups3", name="ups3")
                        o = 0
                        while o < cw:
                            n = min(512, cw - o)
                            nc.tensor.matmul(out=ps3[:, o:o + n], lhsT=cU3[:],
                                             rhs=h2[:, o:o + n], start=True, stop=True)
                            o += n
                        uo = npool.tile([2, CH], F32, tag="uo", name="uo")
                        nc.scalar.copy(out=uo[:, :cw], in_=ps3[:, :cw])
                        nc.sync.dma_start(out=u_stage[r0:r0 + 2, off:off + cw], in_=uo[:, :cw])
                    off += cw

                uem = []
                for r in range(4):
                    t = npool.tile([P, NB], F32, tag=f"uem{r}", name=f"uem{r}")
                    nc.sync.dma_start(out=t[:], in_=u_stage[r, :].rearrange("(p b) -> p b", p=P))
                    uem.append(t)
                U0, USp, UVp, USm = uem
                tt = lambda tg: npool.tile([P, NB], F32, tag=tg, name=tg)
                T = tt("T"); Pm = tt("Pm"); den = tt("den")
                invT = tt("invT"); invC = tt("invC"); invCT = tt("invCT"); Pd2 = tt("Pd2")
                tmp = tt("ntmp"); tmp2 = tt("ntmp2")
                vt_ = nc.vector.tensor_tensor
                vt_(out=tmp[:], in0=USp[:], in1=U0[:], op=TT.subtract)
                nc.scalar.mul(out=T[:], in_=tmp[:], mul=1.0 / EPS_U)
                vt_(out=tmp[:], in0=U0[:], in1=UVp[:], op=TT.subtract)
                nc.scalar.mul(out=Pm[:], in_=tmp[:], mul=1.0 / EPS_U)
                vt_(out=tmp[:], in0=USp[:], in1=USm[:], op=TT.add)
                vt_(out=tmp2[:], in0=U0[:], in1=U0[:], op=TT.add)
                vt_(out=den[:], in0=tmp[:], in1=tmp2[:], op=TT.subtract)
                nc.vector.reciprocal(out=invT[:], in_=T[:])
                vt_(out=tmp[:], in0=den[:], in1=invT[:], op=TT.mult)
                nc.scalar.mul(out=invC[:], in_=tmp[:], mul=1.0 / (EPS_U * EPS_U))
                vt_(out=invCT[:], in0=invC[:], in1=invT[:], op=TT.mult)
                vt_(out=tmp[:], in0=nV[:], in1=nV[:], op=TT.mult)
                vt_(out=Pd2[:], in0=Pm[:], in1=tmp[:], op=TT.mult)
                nvv = npool.tile([P, NB], F32, tag="nvv", name="nvv")
                nc.sync.dma_start(out=nvv[:], in_=vv_half[:])
                eo = tt("eo")
                vt_(out=eo[:], in0=U0[:], in1=nvv[:], op=TT.add)
                nc.sync.dma_start(out=eout[:], in_=eo[:])
                nc.sync.dma_start(out=invT_out[:], in_=invT[:])

                ntab = npool.tile([P, NB * 64], F32, tag="ntab", name="ntab")
                nc.vector.memset(ntab[:], 0.0)
                ntv = ntab[:].rearrange("p (b c) -> p b c", c=64)
                for ci, src in enumerate((T, invT, invCT, invC, Pd2)):
                    nc.vector.tensor_copy(out=ntv[:, :, ci], in_=src[:])
                nvt = npool.tile([P, NB, 3], F32, tag="nvt", name="nvt")
                nc.sync.dma_start(out=nvt[:], in_=v_nm[:])
                for c3 in range(3):
                    nc.vector.tensor_copy(out=ntv[:, :, 5 + c3], in_=nvt[:, :, c3])
                nc.sync.dma_start(out=node_tab[:].rearrange("(p q) -> p q", p=P), in_=ntab[:])

            # ================= edge phase =================
            # acc is Internal (not PJRT zero-donated): zero it on device first
            with tc.tile_pool(name="zpool", bufs=1) as zpool:
                ztile = zpool.tile([P, 4096], F32)
                nc.vector.memset(ztile[:], 0.0)
                accf = acc[:, :].rearrange("r c -> (r c)").rearrange("(p q) -> p q", p=P)
                ACCQ = NR * 64 // P          # 10560 elems per partition
                o = 0
                while o < ACCQ:
                    n = min(4096, ACCQ - o)
                    nc.sync.dma_start(out=accf[:, o:o + n], in_=ztile[:, :n])
                    o += n
            ntab_rows = node_tab[:].rearrange("(r c) -> r c", c=64)
            with tc.tile_pool(name="sbuf", bufs=2) as pool, \
                 tc.tile_pool(name="mlp", bufs=1) as mpool, \
                 tc.tile_pool(name="pA", bufs=2, space="PSUM") as pA, \
                 tc.tile_pool(name="pS", bufs=2, space="PSUM") as pS, \
                 tc.tile_pool(name="pT", bufs=2, space="PSUM") as pT:
                vt = nc.vector.tensor_tensor
                tsc = nc.vector.tensor_scalar
                for sc in range(nsc):
                    ed = pool.tile([P, ET, 14], F32, tag="ed", name="ed")
                    nc.sync.dma_start(out=ed[:], in_=epack[:, sc * ET:(sc + 1) * ET, :])
                    gii = pool.tile([P, 2 * (NI // 16)], I16, tag="gii", name="gii")
                    gji = pool.tile([P, 2 * (NI // 16)], I16, tag="gji", name="gji")
                    sii = pool.tile([P, 2 * (NI // 16)], I16, tag="sii", name="sii")
                    sji = pool.tile([P, 2 * (NI // 16)], I16, tag="sji", name="sji")
                    for t, src in ((gii, gi_idx), (gji, gj_idx), (sii, si_idx), (sji, sj_idx)):
                        nc.sync.dma_start(out=t[:], in_=src[sc])
                    gi = pool.tile([P, ET, 64], F32, tag="gi", name="gi")
                    gj = pool.tile([P, ET, 64], F32, tag="gj", name="gj")
                    for gt, it in ((gi, gii), (gj, gji)):
                        for hf in range(2):
                            nc.gpsimd.dma_gather(
                                out_ap=gt[:, hf * (ET // 2):(hf + 1) * (ET // 2), :],
                                in_ap=ntab_rows,
                                idxs_ap=it[:, hf * (NI // 16):(hf + 1) * (NI // 16)],
                                num_idxs=NI, num_idxs_reg=NI, elem_size=64)

                    g = lambda tg: pool.tile([P, ET], F32, tag=tg, name=tg)
                    vij = [g(f"vij{c}") for c in range(3)]
                    for c in range(3):
                        vt(out=vij[c][:], in0=gi[:, :, 5 + c], in1=gj[:, :, 5 + c], op=TT.subtract)
                    r2 = g("r2"); tmpe = g("tmpe"); tmpf = g("tmpf")
                    vt(out=r2[:], in0=ed[:, :, 0], in1=ed[:, :, 0], op=TT.mult)
                    for c in (1, 2):
                        vt(out=tmpe[:], in0=ed[:, :, c], in1=ed[:, :, c], op=TT.mult)
                        vt(out=r2[:], in0=r2[:], in1=tmpe[:], op=TT.add)
                    rpl = g("rpl")
                    nc.scalar.activation(out=rpl[:], in_=r2[:], func=AF.Sqrt)
                    rinv = g("rinv")
                    tsc(out=rinv[:], in0=rpl[:], scalar1=1e-8, scalar2=None, op0=TT.add)
                    nc.vector.reciprocal(out=rinv[:], in_=rinv[:])
                    epl = [g(f"e{c}") for c in range(3)]
                    for c in range(3):
                        vt(out=epl[c][:], in0=ed[:, :, c], in1=rinv[:], op=TT.mult)
                    ev = g("ev"); vv = g("vv")
                    vt(out=ev[:], in0=epl[0][:], in1=vij[0][:], op=TT.mult)
                    vt(out=vv[:], in0=vij[0][:], in1=vij[0][:], op=TT.mult)
                    for c in (1, 2):
                        vt(out=tmpe[:], in0=epl[c][:], in1=vij[c][:], op=TT.mult)
                        vt(out=ev[:], in0=ev[:], in1=tmpe[:], op=TT.add)
                        vt(out=tmpe[:], in0=vij[c][:], in1=vij[c][:], op=TT.mult)
                        vt(out=vv[:], in0=vv[:], in1=tmpe[:], op=TT.add)

                    stg = pool.tile([P, ET, 4], F32, tag="stg", name="stg")
                    nc.vector.tensor_copy(out=stg[:, :, 0], in_=rpl[:])
                    nc.vector.tensor_copy(out=stg[:, :, 1], in_=gi[:, :, 0])
                    nc.vector.tensor_copy(out=stg[:, :, 2], in_=gj[:, :, 0])
                    nc.vector.memset(stg[:, :, 3], 0.0)
                    xt = mpool.tile([4, ET * P], F32, tag="xt", name="xt")
                    for t in range(ET):
                        pst = pT.tile([P, P], F32, tag="tp", name="tp")
                        nc.tensor.transpose(out=pst[:4, :], in_=stg[:, t, :], identity=ident[:])
                        nc.vector.tensor_copy(out=xt[:, t * P:(t + 1) * P], in_=pst[:4, :])

                    # l1: per half, 4 stationaries, silu x2 bias variants
                    h1 = {k: (mpool.tile([P, SC], F32, tag=f"h1{k}0", name=f"h1{k}0"),
                              mpool.tile([P, SC], F32, tag=f"h1{k}1", name=f"h1{k}1"))
                          for k in ("ABi", "ABj", "CWi", "CWj")}
                    for hf in range(2):
                        for key in ("ABi", "ABj", "CWi", "CWj"):
                            bt = cB1["AB" if key.startswith("AB") else "CW"]
                            psl = pA.tile([P, HSC], F32, tag="A", name="A")
                            for t8 in range(8):
                                t = hf * 8 + t8
                                nc.tensor.matmul(out=psl[:, t8 * P:(t8 + 1) * P],
                                                 lhsT=cL1[key][:],
                                                 rhs=xt[:, t * P:(t + 1) * P],
                                                 start=True, stop=True)
                            sl = slice(hf * HSC, (hf + 1) * HSC)
                            nc.scalar.activation(out=h1[key][0][:, sl], in_=psl[:], func=AF.Silu, bias=bt[:, 0:1])
                            nc.scalar.activation(out=h1[key][1][:, sl], in_=psl[:], func=AF.Silu, bias=bt[:, 1:2])

                    # l2 + l3 stacks (per side, per half)
                    sv_i = mpool.tile([P, SC], F32, tag="sv_i", name="sv_i")
                    sv_j = mpool.tile([P, SC], F32, tag="sv_j", name="sv_j")
                    for side, svt in (("i", sv_i), ("j", sv_j)):
                        for hf in range(2):
                            stks = [pS.tile([P, 512], F32, tag="S", name="S") for _ in range(2)]
                            for vi, (key, var) in enumerate(((f"AB{side}", 0), (f"AB{side}", 1),
                                                            (f"CW{side}", 0), (f"CW{side}", 1))):
                                nm2 = "AB" if key.startswith("AB") else "CW"
                                hin = h1[key][var]
                                ps2 = pA.tile([P, HSC], F32, tag="A", name="A")
                                for o in (0, 512):
                                    nc.tensor.matmul(out=ps2[:, o:o + 512], lhsT=cL2[nm2][:],
                                                     rhs=hin[:, hf * HSC + o:hf * HSC + o + 512],
                                                     start=True, stop=True)
                                h2 = mpool.tile([P, HSC], F32, tag="h2", name="h2")
                                nc.scalar.activation(out=h2[:], in_=ps2[:], func=AF.Silu, bias=cB2[nm2][:, 0:1])
                                for ci in range(2):
                                    nc.tensor.matmul(out=stks[ci][32 * vi:32 * (vi + 1), :],
                                                     lhsT=cL3[nm2][:], rhs=h2[:, ci * 512:(ci + 1) * 512],
                                                     start=True, stop=True,
                                                     tile_position=(0, 32 * vi))
                            for ci in range(2):
                                nc.vector.tensor_copy(out=svt[:, hf * HSC + ci * 512:hf * HSC + (ci + 1) * 512],
                                                      in_=stks[ci][:])

                    vem_i = pool.tile([P, ET, P], F32, tag="vem_i", name="vem_i")
                    vem_j = pool.tile([P, ET, P], F32, tag="vem_j", name="vem_j")
                    for svt, vem in ((sv_i, vem_i), (sv_j, vem_j)):
                        for t in range(ET):
                            pst = pT.tile([P, P], F32, tag="tp", name="tp")
                            nc.tensor.transpose(out=pst[:], in_=svt[:, t * P:(t + 1) * P], identity=ident[:])
                            nc.vector.tensor_copy(out=vem[:, t, :], in_=pst[:])

                    # ---- payload combine ----
                    pb = g
                    def mkval(dst, vem, v, r, bias):
                        tsc(out=dst[:], in0=vem[:, :, 32 * v + r], scalar1=float(bias), scalar2=None, op0=TT.add)
                    A_i = pb("A_i"); B_i = pb("B_i"); A_ie = pb("A_ie"); B_ie = pb("B_ie")
                    C_i = pb("C_i"); Wp = pb("Wp"); C_ie = pb("C_ie"); Wm = pb("Wm")
                    A_j = pb("A_j"); B_j = pb("B_j"); A_je = pb("A_je"); B_je = pb("B_je")
                    C_j = pb("C_j"); C_je = pb("C_je")
                    mkval(A_i, vem_i, 0, 0, b3['A']); mkval(B_i, vem_i, 0, 1, b3['B'])
                    mkval(A_ie, vem_i, 1, 0, b3['A']); mkval(B_ie, vem_i, 1, 1, b3['B'])
                    mkval(C_i, vem_i, 2, 0, b3['C']); mkval(C_ie, vem_i, 3, 0, b3['C'])
                    mkval(A_j, vem_j, 0, 0, b3['A']); mkval(B_j, vem_j, 0, 1, b3['B'])
                    mkval(A_je, vem_j, 1, 0, b3['A']); mkval(C_j, vem_j, 2, 0, b3['C'])
                    mkval(B_je, vem_j, 1, 1, b3['B']); mkval(C_je, vem_j, 3, 0, b3['C'])
                    for dst, v, sgn in ((Wp, 2, 1.0), (Wm, 3, -1.0)):
                        tsc(out=tmpe[:], in0=vem_i[:, :, 32 * v + 1], scalar1=float(b3['W']), scalar2=None, op0=TT.add)
                        nc.scalar.activation(out=tmpe[:], in_=tmpe[:], func=AF.Exp)
                        tsc(out=tmpf[:], in0=rpl[:], scalar1=sgn * EPS_T / H, scalar2=None, op0=TT.add)
                        vt(out=tmpf[:], in0=tmpf[:], in1=tmpf[:], op=TT.mult)
                        tsc(out=tmpf[:], in0=tmpf[:], scalar1=-1.0, scalar2=1.0, op0=TT.mult, op1=TT.add)
                        vt(out=dst[:], in0=tmpe[:], in1=tmpf[:], op=TT.mult)
                    dWdr = pb("dWdr")
                    vt(out=dWdr[:], in0=Wp[:], in1=Wm[:], op=TT.subtract)
                    tsc(out=dWdr[:], in0=dWdr[:], scalar1=1.0 / (2 * EPS_T), scalar2=None, op0=TT.mult)

                    A_ij = pb("A_ij"); B_ij = pb("B_ij"); C_ij = pb("C_ij")
                    vt(out=A_ij[:], in0=A_i[:], in1=A_j[:], op=TT.mult)
                    vt(out=B_ij[:], in0=B_i[:], in1=B_j[:], op=TT.mult)
                    vt(out=C_ij[:], in0=C_i[:], in1=C_j[:], op=TT.mult)

                    def grad(dst, Pij, Xe, Xo):
                        vt(out=tmpe[:], in0=Xe[:], in1=Xo[:], op=TT.mult)
                        vt(out=tmpe[:], in0=tmpe[:], in1=Pij[:], op=TT.subtract)
                        vt(out=tmpe[:], in0=tmpe[:], in1=Pij[:], op=TT.mult)
                        tsc(out=dst[:], in0=tmpe[:], scalar1=2.0 / EPS_T, scalar2=None, op0=TT.mult)
                    gA_i = pb("gA_i"); gB_i = pb("gB_i"); gC_i = pb("gC_i")
                    gA_j = pb("gA_j"); gB_j = pb("gB_j"); gC_j = pb("gC_j")
                    grad(gA_i, A_ij, A_ie, A_j); grad(gB_i, B_ij, B_ie, B_j)
                    grad(gC_i, C_ij, C_ie, C_j)
                    grad(gA_j, A_ij, A_je, A_i); grad(gB_j, B_ij, B_je, B_i)
                    grad(gC_j, C_ij, C_je, C_i)

                    Ti = gi[:, :, 0]; invTi = gi[:, :, 1]; invCiTi = gi[:, :, 2]
                    invCi = gi[:, :, 3]; Pd2i = gi[:, :, 4]
                    Tj = gj[:, :, 0]; invTj = gj[:, :, 1]; invCjTj = gj[:, :, 2]
                    invCj = gj[:, :, 3]; Pd2j = gj[:, :, 4]

                    sTin = pb("sTin"); sCTin = pb("sCTin")
                    vt(out=sTin[:], in0=invTi, in1=invTj, op=TT.add)
                    vt(out=sCTin[:], in0=invCiTi, in1=invCjTj, op=TT.add)
                    a2h = pb("a2h"); coef = pb("coef")
                    vt(out=tmpe[:], in0=A_ij[:], in1=A_ij[:], op=TT.mult)
                    tsc(out=a2h[:], in0=tmpe[:], scalar1=0.5, scalar2=None, op0=TT.mult)
                    vt(out=tmpf[:], in0=B_ij[:], in1=B_ij[:], op=TT.mult)
                    vt(out=tmpf[:], in0=tmpf[:], in1=tmpe[:], op=TT.subtract)
                    tsc(out=tmpf[:], in0=tmpf[:], scalar1=1.0 / D, scalar2=None, op0=TT.mult)
                    vt(out=coef[:], in0=a2h[:], in1=tmpf[:], op=TT.add)
                    term6 = pb("term6")
                    tsc(out=term6[:], in0=a2h[:], scalar1=float(D), scalar2=None, op0=TT.mult)
                    vt(out=term6[:], in0=term6[:], in1=coef[:], op=TT.add)
                    tsc(out=term6[:], in0=term6[:], scalar1=-1.0, scalar2=None, op0=TT.mult)

                    def gcoef(dst, gA, gB):
                        vt(out=tmpe[:], in0=gB[:], in1=gA[:], op=TT.subtract)
                        tsc(out=tmpe[:], in0=tmpe[:], scalar1=1.0 / D, scalar2=None, op0=TT.mult)
                        tsc(out=tmpf[:], in0=gA[:], scalar1=0.5, scalar2=None, op0=TT.mult)
                        vt(out=dst[:], in0=tmpf[:], in1=tmpe[:], op=TT.add)
                    gco_i = pb("gco_i"); gco_j = pb("gco_j")
                    gcoef(gco_i, gA_i, gB_i); gcoef(gco_j, gA_j, gB_j)
                    gAh_i = pb("gAh_i"); gAh_j = pb("gAh_j")
                    tsc(out=gAh_i[:], in0=gA_i[:], scalar1=0.5, scalar2=None, op0=TT.mult)
                    tsc(out=gAh_j[:], in0=gA_j[:], scalar1=0.5, scalar2=None, op0=TT.mult)

                    tr = pb("tr"); trD = pb("trD")
                    vt(out=tr[:], in0=ed[:, :, 3], in1=ed[:, :, 7], op=TT.add)
                    vt(out=tr[:], in0=tr[:], in1=ed[:, :, 11], op=TT.add)
                    tsc(out=trD[:], in0=tr[:], scalar1=1.0 / D, scalar2=None, op0=TT.mult)
                    termw = [pb(f"tw{c}") for c in range(3)]
                    for a in range(3):
                        for bb in range(3):
                            vt(out=tmpe[:], in0=ed[:, :, 3 + 3 * a + bb], in1=ed[:, :, 3 + 3 * bb + a], op=TT.add)
                            tsc(out=tmpe[:], in0=tmpe[:], scalar1=0.5, scalar2=None, op0=TT.mult)
                            if a == bb:
                                vt(out=tmpe[:], in0=tmpe[:], in1=trD[:], op=TT.subtract)
                            vt(out=tmpe[:], in0=tmpe[:], in1=A_ij[:], op=TT.mult)
                            if a == bb:
                                vt(out=tmpf[:], in0=B_ij[:], in1=trD[:], op=TT.mult)
                                vt(out=tmpe[:], in0=tmpe[:], in1=tmpf[:], op=TT.add)
                            vt(out=tmpe[:], in0=tmpe[:], in1=epl[bb][:], op=TT.mult)
                            if bb == 0:
                                nc.vector.tensor_copy(out=termw[a][:], in_=tmpe[:])
                            else:
                                vt(out=termw[a][:], in0=termw[a][:], in1=tmpe[:], op=TT.add)
                    termSw = pb("termSw")
                    vt(out=termSw[:], in0=termw[0][:], in1=vij[0][:], op=TT.mult)
                    for c in (1, 2):
                        vt(out=tmpe[:], in0=termw[c][:], in1=vij[c][:], op=TT.mult)
                        vt(out=termSw[:], in0=termSw[:], in1=tmpe[:], op=TT.add)
                    tsc(out=termSw[:], in0=termSw[:], scalar1=-0.5, scalar2=None, op0=TT.mult)

                    sPd = pb("sPd")
                    vt(out=sPd[:], in0=Pd2i, in1=Pd2j, op=TT.add)
                    vt(out=sPd[:], in0=sPd[:], in1=dWdr[:], op=TT.mult)
                    stt = pb("stt")
                    vt(out=stt[:], in0=sTin[:], in1=sCTin[:], op=TT.subtract)
                    cvij = pb("cvij")
                    vt(out=cvij[:], in0=stt[:], in1=a2h[:], op=TT.mult)
                    vt(out=tmpe[:], in0=gAh_i[:], in1=invCi, op=TT.mult)
                    vt(out=cvij[:], in0=cvij[:], in1=tmpe[:], op=TT.add)
                    vt(out=tmpe[:], in0=gAh_j[:], in1=invCj, op=TT.mult)
                    vt(out=cvij[:], in0=cvij[:], in1=tmpe[:], op=TT.add)
                    ce = pb("ce")
                    vt(out=ce[:], in0=stt[:], in1=coef[:], op=TT.mult)
                    vt(out=tmpe[:], in0=gco_i[:], in1=invCi, op=TT.mult)
                    vt(out=ce[:], in0=ce[:], in1=tmpe[:], op=TT.add)
                    vt(out=tmpe[:], in0=gco_j[:], in1=invCj, op=TT.mult)
                    vt(out=ce[:], in0=ce[:], in1=tmpe[:], op=TT.add)
                    vt(out=ce[:], in0=ce[:], in1=ev[:], op=TT.mult)
                    wm = ed[:, :, 13]
                    u3 = [pb(f"u3{c}") for c in range(3)]
                    for c in range(3):
                        vt(out=tmpe[:], in0=cvij[:], in1=vij[c][:], op=TT.mult)
                        vt(out=tmpf[:], in0=ce[:], in1=epl[c][:], op=TT.mult)
                        vt(out=tmpe[:], in0=tmpe[:], in1=tmpf[:], op=TT.add)
                        tsc(out=tmpe[:], in0=tmpe[:], scalar1=-0.5, scalar2=None, op0=TT.mult)
                        vt(out=tmpf[:], in0=sPd[:], in1=epl[c][:], op=TT.mult)
                        vt(out=tmpe[:], in0=tmpe[:], in1=tmpf[:], op=TT.subtract)
                        tsc(out=tmpf[:], in0=termw[c][:], scalar1=CW_COEF, scalar2=None, op0=TT.mult)
                        vt(out=tmpe[:], in0=tmpe[:], in1=tmpf[:], op=TT.add)
                        vt(out=u3[c][:], in0=tmpe[:], in1=wm, op=TT.mult)

                    aux2 = pb("aux2"); ev2 = pb("ev2")
                    vt(out=ev2[:], in0=ev[:], in1=ev[:], op=TT.mult)
                    vt(out=aux2[:], in0=a2h[:], in1=vv[:], op=TT.mult)
                    vt(out=tmpf[:], in0=coef[:], in1=ev2[:], op=TT.mult)
                    vt(out=aux2[:], in0=aux2[:], in1=tmpf[:], op=TT.add)
                    tsc(out=aux2[:], in0=aux2[:], scalar1=0.25, scalar2=None, op0=TT.mult)
                    C2 = pb("C2")
                    vt(out=C2[:], in0=C_ij[:], in1=C_ij[:], op=TT.mult)
                    dTin = pb("dTin")
                    vt(out=dTin[:], in0=invTi, in1=invTj, op=TT.subtract)
                    sm1 = pb("sm1"); sm2 = pb("sm2")
                    vt(out=tmpe[:], in0=sTin[:], in1=aux2[:], op=TT.mult)
                    vt(out=tmpf[:], in0=dTin[:], in1=C2[:], op=TT.mult)
                    vt(out=sm1[:], in0=tmpe[:], in1=tmpf[:], op=TT.add)
                    vt(out=sm2[:], in0=tmpe[:], in1=tmpf[:], op=TT.subtract)
                    two_i = pb("two_i"); t1S = pb("t1S"); t4 = pb("t4")
                    tsc(out=two_i[:], in0=invCiTi, scalar1=2.0, scalar2=None, op0=TT.mult)
                    vt(out=tmpe[:], in0=two_i[:], in1=invCjTj, op=TT.add)
                    vt(out=t1S[:], in0=tmpe[:], in1=aux2[:], op=TT.mult)
                    tsc(out=t1S[:], in0=t1S[:], scalar1=-1.0, scalar2=None, op0=TT.mult)
                    vt(out=tmpe[:], in0=two_i[:], in1=invCjTj, op=TT.subtract)
                    vt(out=t4[:], in0=tmpe[:], in1=C2[:], op=TT.mult)
                    tsc(out=t4[:], in0=t4[:], scalar1=-1.0, scalar2=None, op0=TT.mult)
                    t2S = pb("t2S")
                    vt(out=tmpe[:], in0=gAh_i[:], in1=vv[:], op=TT.mult)
                    vt(out=tmpf[:], in0=gco_i[:], in1=ev2[:], op=TT.mult)
                    vt(out=tmpe[:], in0=tmpe[:], in1=tmpf[:], op=TT.add)
                    vt(out=t2S[:], in0=tmpe[:], in1=invCi, op=TT.mult)
                    vt(out=tmpe[:], in0=gAh_j[:], in1=vv[:], op=TT.mult)
                    vt(out=tmpf[:], in0=gco_j[:], in1=ev2[:], op=TT.mult)
                    vt(out=tmpe[:], in0=tmpe[:], in1=tmpf[:], op=TT.add)
                    vt(out=tmpe[:], in0=tmpe[:], in1=invCj, op=TT.mult)
                    vt(out=t2S[:], in0=t2S[:], in1=tmpe[:], op=TT.add)
                    tsc(out=t2S[:], in0=t2S[:], scalar1=0.25, scalar2=None, op0=TT.mult)
                    t5 = pb("t5")
                    vt(out=t5[:], in0=gC_i[:], in1=invCi, op=TT.mult)
                    vt(out=tmpe[:], in0=gC_j[:], in1=invCj, op=TT.mult)
                    vt(out=t5[:], in0=t5[:], in1=tmpe[:], op=TT.subtract)
                    com = pb("com")
                    vt(out=com[:], in0=t1S[:], in1=t2S[:], op=TT.add)
                    vt(out=com[:], in0=com[:], in1=term6[:], op=TT.add)
                    cdv = pb("cdv")
                    vt(out=cdv[:], in0=C_ij[:], in1=ed[:, :, 12], op=TT.mult)
                    si = pb("si"); sj = pb("sj")
                    vt(out=si[:], in0=sm1[:], in1=com[:], op=TT.add)
                    vt(out=si[:], in0=si[:], in1=t4[:], op=TT.add)
                    vt(out=si[:], in0=si[:], in1=t5[:], op=TT.add)
                    vt(out=tmpe[:], in0=termSw[:], in1=cdv[:], op=TT.add)
                    tsc(out=tmpe[:], in0=tmpe[:], scalar1=CW_COEF, scalar2=None, op0=TT.mult)
                    vt(out=si[:], in0=si[:], in1=tmpe[:], op=TT.add)
                    vt(out=si[:], in0=si[:], in1=wm, op=TT.mult)
                    vt(out=sj[:], in0=sm2[:], in1=com[:], op=TT.add)
                    vt(out=sj[:], in0=sj[:], in1=t4[:], op=TT.subtract)
                    vt(out=sj[:], in0=sj[:], in1=t5[:], op=TT.subtract)
                    vt(out=tmpe[:], in0=termSw[:], in1=cdv[:], op=TT.subtract)
                    tsc(out=tmpe[:], in0=tmpe[:], scalar1=CW_COEF, scalar2=None, op0=TT.mult)
                    vt(out=sj[:], in0=sj[:], in1=tmpe[:], op=TT.add)
                    vt(out=sj[:], in0=sj[:], in1=wm, op=TT.mult)

                    qi = pool.tile([P, ET, 8], F32, tag="qi", name="qi")
                    qj = pool.tile([P, ET, 8], F32, tag="qj", name="qj")
                    nc.vector.memset(qi[:], 0.0)
                    nc.vector.memset(qj[:], 0.0)
                    for c in range(3):
                        nc.vector.tensor_copy(out=qi[:, :, c], in_=u3[c][:])
                        tsc(out=qj[:, :, c], in0=u3[c][:], scalar1=-1.0, scalar2=None, op0=TT.mult)
                    nc.vector.tensor_copy(out=qi[:, :, 3], in_=si[:])
                    nc.vector.tensor_copy(out=qj[:, :, 3], in_=sj[:])
                    for qt, it in ((qi, sii), (qj, sji)):
                        for hf in range(2):
                            nc.gpsimd.dma_scatter_add(
                                out_ap=acc[:, :8],
                                in_ap=qt[:, hf * (ET // 2):(hf + 1) * (ET // 2), :],
                                idxs_ap=it[:, hf * (NI // 16):(hf + 1) * (NI // 16)],
                                num_idxs=NI, num_idxs_reg=NI, elem_size=8, elem_step=64)

            with tc.tile_pool(name="fin", bufs=1) as fpool:
                accs = fpool.tile([P, NB, 8], F32)
                nc.sync.dma_start(out=accs[:], in_=acc[:NP, :8].rearrange("(p b) c -> p b c", p=P))
                nc.sync.dma_start(out=acc_out[:, :].rearrange("(p b) c -> p b c", p=P), in_=accs[:])

    nc.compile()
    return nc


def _prep_host(v, edge_index, r_ij, S, d, dW, dV, params):
    f32 = lambda x: np.asarray(x, np.float32)
    pr = {}
    for nm in ('W', 'A', 'B', 'C'):
        pr[nm] = [(f32(w), f32(b)) for (w, b) in params[nm]]
    pr['U'] = [(np.abs(f32(w)), f32(b)) for (w, b) in params['U']]

    i_all = np.asarray(edge_index[0], np.int64)
    j_all = np.asarray(edge_index[1], np.int64)

    cores = []
    slots_per_core = 0
    for c in range(NCORES):
        sl = slice(c * EPC, (c + 1) * EPC)
        groups = _group_edges(i_all[sl], j_all[sl])
        tok = sum((len(g) + NI - 1) // NI * NI for g in groups)
        slots_per_core = max(slots_per_core, tok)
        cores.append((sl, groups))
    nsc = (slots_per_core + SC - 1) // SC
    slots = nsc * SC

    pidx, bidx = np.meshgrid(np.arange(P), np.arange(NB), indexing='ij')
    nn = 128 * bidx + pidx
    valid = nn < N
    nnc = np.where(valid, nn, 0)
    S_nm = np.where(valid, f32(S)[nnc, 0], 0.0).astype(np.float32)
    d_nm = np.where(valid, f32(d)[nnc, 0], 1.0).astype(np.float32)
    v_npv = f32(v)
    v_nm = np.where(valid[..., None], v_npv[nnc], 0.0).astype(np.float32)
    vv_half = 0.5 * (v_nm ** 2).sum(-1).astype(np.float32)

    r_np = f32(r_ij)
    dW_np = f32(dW).reshape(E, 9)
    dV_np = f32(dV)

    ins = []
    for c in range(NCORES):
        sl, groups = cores[c]
        base = sl.start
        order = np.full(slots, -1, np.int64)
        pos = 0
        for gids in groups:
            order[pos:pos + len(gids)] = base + gids
            pos += (len(gids) + NI - 1) // NI * NI
        pad = order < 0
        oc = np.where(pad, 0, order)

        ep = np.zeros((slots, 14), np.float32)
        ep[:, 0:3] = r_np[oc]
        ep[:, 3:12] = dW_np[oc]
        ep[:, 12] = dV_np[oc, 0]
        ep[:, 13] = 1.0
        ep[pad, :] = 0.0
        ep[pad, 0] = 1.0
        epack = ep.reshape(slots // P, P, 14).transpose(1, 0, 2).copy()

        gi_t = np.where(pad, 0, _remap(i_all[oc]))
        gj_t = np.where(pad, 0, _remap(j_all[oc]))
        trash = NP + (np.arange(slots) % NI)
        si_t = np.where(pad, trash, _remap(i_all[oc]))
        sj_t = np.where(pad, trash, _remap(j_all[oc]))

        def wrap_calls(tokens):
            out = np.zeros((nsc, P, 2 * (NI // 16)), np.int16)
            for s in range(nsc):
                for h in range(2):
                    t0 = s * SC + h * NI
                    out[s, :, h * (NI // 16):(h + 1) * (NI // 16)] = _wrap16(tokens[t0:t0 + NI])
            return out

        ins.append({
            "S_nm": S_nm, "d_nm": d_nm, "v_nm": v_nm, "vv_half": vv_half,
            "epack": epack,
            "gi_idx": wrap_calls(gi_t), "gj_idx": wrap_calls(gj_t),
            "si_idx": wrap_calls(si_t), "sj_idx": wrap_calls(sj_t),
        })

    W1 = {nm: pr[nm][0] for nm in 'ABCW'}
    W2 = {nm: pr[nm][1] for nm in 'ABCW'}
    W3 = {nm: pr[nm][2] for nm in 'ABCW'}

    def l1_stat(side):
        ab = np.zeros((4, P), np.float32)
        cw = np.zeros((4, P), np.float32)
        trow = 1 if side == 'i' else 2
        for h, nm in ((0, 'A'), (1, 'B')):
            w = W1[nm][0]
            ab[0, h * 64:(h + 1) * 64] = w[0]
            ab[trow, h * 64:(h + 1) * 64] = w[1]
        cw[0, 0:64] = W1['C'][0][0]
        cw[trow, 0:64] = W1['C'][0][1]
        cw[0, 64:128] = W1['W'][0][0] / H
        return ab, cw

    L1_ABi_, L1_CWi_ = l1_stat('i')
    L1_ABj_, L1_CWj_ = l1_stat('j')
    z64 = np.zeros((64, 64), np.float32)
    blk = lambda a, b: np.block([[a, z64], [z64, b]]).astype(np.float32)
    L2_AB_ = blk(W2['A'][0], W2['B'][0])
    L2_CW_ = blk(W2['C'][0], W2['W'][0])
    L3_AB_ = np.zeros((P, 32), np.float32)
    L3_AB_[0:64, 0] = W3['A'][0][:, 0]; L3_AB_[64:128, 1] = W3['B'][0][:, 0]
    L3_CW_ = np.zeros((P, 32), np.float32)
    L3_CW_[0:64, 0] = W3['C'][0][:, 0]; L3_CW_[64:128, 1] = W3['W'][0][:, 0]

    B1_AB_ = np.zeros((P, 2), np.float32)
    B1_AB_[0:64, 0] = W1['A'][1]; B1_AB_[64:128, 0] = W1['B'][1]
    B1_AB_[0:64, 1] = W1['A'][1] + EPS_T * W1['A'][0][1]
    B1_AB_[64:128, 1] = W1['B'][1] + EPS_T * W1['B'][0][1]
    B1_CW_ = np.zeros((P, 2), np.float32)
    B1_CW_[0:64, 0] = W1['C'][1]
    B1_CW_[64:128, 0] = W1['W'][1] + (EPS_T / H) * W1['W'][0][0]
    B1_CW_[0:64, 1] = W1['C'][1] + EPS_T * W1['C'][0][1]
    B1_CW_[64:128, 1] = W1['W'][1] - (EPS_T / H) * W1['W'][0][0]
    B2_AB_ = np.concatenate([W2['A'][1], W2['B'][1]]).reshape(P, 1).astype(np.float32)
    B2_CW_ = np.concatenate([W2['C'][1], W2['W'][1]]).reshape(P, 1).astype(np.float32)
    b3 = {nm: float(W3[nm][1][0]) for nm in 'ABCW'}

    u1 = pr['U'][0][0]
    U1_ = np.stack([u1[0], -u1[1]]).astype(np.float32)
    U2_ = blk(pr['U'][1][0], pr['U'][1][0])
    U3_ = np.zeros((P, 2), np.float32)
    U3_[0:64, 0] = pr['U'][2][0][:, 0]; U3_[64:128, 1] = pr['U'][2][0][:, 0]
    b0 = pr['U'][0][1]
    Ub1v_ = np.stack([b0, b0 + EPS_U * u1[0], b0 - EPS_U * u1[1], b0 - EPS_U * u1[0]], 1).astype(np.float32)
    Ub2_ = np.concatenate([pr['U'][1][1], pr['U'][1][1]]).reshape(P, 1).astype(np.float32)
    b2U = float(pr['U'][2][1][0])

    wts = {
        "U1": U1_, "U2": U2_, "U3": U3_, "Ub1v": Ub1v_, "Ub2": Ub2_,
        "L1_ABi": L1_ABi_, "L1_ABj": L1_ABj_, "L1_CWi": L1_CWi_, "L1_CWj": L1_CWj_,
        "L2_AB": L2_AB_, "L2_CW": L2_CW_, "L3_AB": L3_AB_, "L3_CW": L3_CW_,
        "B1_AB": B1_AB_, "B1_CW": B1_CW_, "B2_AB": B2_AB_, "B2_CW": B2_CW_,
    }
    for m in ins:
        m.update(wts)
    return ins, nsc, b3, b2U


def _make_runner(nc):
    """Build the PJRT multi-core executable once (run_bass_via_pjrt re-traces
    and re-jits on every call; this caches the sharded callable)."""
    import jax
    import concourse.mybir as mb
    from concourse import bass2jax
    bass2jax.install_neuronx_cc_hook()
    in_names, out_names, out_avals = [], [], []
    pname = nc.partition_id_tensor.name if nc.partition_id_tensor else None
    for alloc in nc.m.functions[0].allocations:
        if not isinstance(alloc, mb.MemoryLocationSet):
            continue
        name = alloc.memorylocations[0].name
        if alloc.kind == "ExternalInput":
            if name != pname:
                in_names.append(name)
        elif alloc.kind == "ExternalOutput":
            out_names.append(name)
            out_avals.append(jax.core.ShapedArray(tuple(alloc.tensor_shape),
                                                  mb.dt.np(alloc.dtype)))
    n_params = len(in_names)
    all_in = in_names + out_names + ([pname] if pname else [])
    donate = tuple(range(n_params, n_params + len(out_names)))

    def _body(*args):
        operands = list(args)
        if pname is not None:
            operands.append(bass2jax.partition_id_tensor())
        return tuple(bass2jax._bass_exec_p.bind(
            *operands, out_avals=tuple(out_avals), in_names=tuple(all_in),
            out_names=tuple(out_names), lowering_input_output_aliases=(),
            sim_require_finite=True, sim_require_nnan=True, nc=nc))

    devices = jax.devices()[:NCORES]
    mesh = bass2jax.Mesh(np.asarray(devices), ("core",))
    specs = (bass2jax.PartitionSpec("core"),) * (n_params + len(out_names))
    sharded = jax.jit(bass2jax.shard_map(_body, mesh=mesh, in_specs=specs,
                                         out_specs=(bass2jax.PartitionSpec("core"),) * len(out_names),
                                         check_rep=False),
                      donate_argnums=donate, keep_unused=True)

    def run(in_maps):
        concat = [np.concatenate([np.asarray(m[nm]) for m in in_maps], axis=0)
                  for nm in in_names]
        zeros = [np.zeros((NCORES * a.shape[0], *a.shape[1:]), a.dtype)
                 for a in out_avals]
        outs = sharded(*concat, *zeros)
        return [{nm: np.asarray(outs[k]).reshape(NCORES, *out_avals[k].shape)[c]
                 for k, nm in enumerate(out_names)} for c in range(NCORES)]

    return run


def kernel(v, edge_index, r_ij, S, d, dW, dV, params):
    ins, nsc, b3, b2U = _prep_host(v, edge_index, r_ij, S, d, dW, dV, params)
    key = (nsc, tuple(sorted(b3.items())))
    if key not in _PROG_CACHE:
        nc = _build_program(nsc, b3)
        _PROG_CACHE[key] = (nc, _make_runner(nc))
    nc, runner = _PROG_CACHE[key]
    results = runner(ins)

    class _Res:
        pass
    res = _Res()
    res.results = results

    acc = np.zeros((NP, 8), np.float64)
    for c in range(NCORES):
        acc += res.results[c]["acc_out"]
    acc = acc.astype(np.float32)
    rows = _remap(np.arange(N))
    invT = res.results[0]["invT_out"]
    eoutv = res.results[0]["eout"]
    n_p = np.arange(N) % P
    n_b = np.arange(N) // P
    out0 = acc[rows, 0:3].astype(np.float32)
    out1 = (acc[rows, 3] * invT[n_p, n_b])[:, None].astype(np.float32)
    out2 = (eoutv[n_p, n_b][:, None] + b2U).astype(np.float32)
    return out0, out1, out2


# revision 12
# speedup vs baseline: 1.0714x; 1.0714x over previous
"""Trainium2 Bass kernel for nn_CG_model (GNN message passing).

Edge parallelism across 8 NeuronCores (per sharding hint): each core holds the
full node set, processes E/8 edges, and produces a partial N-sized scatter
accumulator; the host sums the 8 partials (the all-reduce) and applies the
final per-node 1/T scale.

Device-side structure (per core):
 - Node phase: MonotonicMLP U-net; the four (S,V) finite-difference variants
   ride one shared matmul with per-partition ACT bias shifts. Produces
   T, 1/T, 1/(C T), 1/C, P/d^2 and writes a 256B-row node table to DRAM
   (partition-major remapped node ids so table writes are contiguous).
 - Edge phase in superchunks of 2048 edges: dma_gather (HW token gather,
   1024 idx/call) pulls i- and j-node channels edge-major; geometry and the
   whole payload combine run on DVE edge-major planes; the A/B/C/W MLPs run
   feature-major on the PE with T+eps / r+-eps variants as ACT bias shifts;
   l3 results are written at 32-row PSUM bases, PE-transposed back to
   edge-major; dma_scatter_add pushes 8-channel payloads (u3, si / -u3, sj)
   into a 256B-strided accumulator.
 - dma_scatter_add drops colliding updates, so the host pre-groups edges into
   1024-token windows with all-distinct i and all-distinct j; window padding
   targets unique trash rows past the node range. Calls serialize on the
   accumulator WAW chain.
"""

import numpy as np

import concourse.bass as bass
import concourse.bacc as bacc
import concourse.tile as tile
import concourse.mybir as mybir
from concourse import bass_utils
from concourse.masks import make_identity

F32 = mybir.dt.float32
I16 = mybir.dt.int16
AF = mybir.ActivationFunctionType
TT = mybir.AluOpType

# problem constants (hardcoded per harness contract)
N, E, D, H, DT, HID = 20000, 640000, 3, 1.0, 0.01, 64
NCORES = 8
EPC = E // NCORES
P = 128
NB = 157                     # node blocks; NP = 128*157 = 20096 >= N
NP = P * NB
NI = 1024                    # tokens per gather/scatter call
ET = 16                      # edge tiles per superchunk
SC = P * ET                  # 2048 edges per superchunk
HSC = SC // 2                # 1024 (half superchunk = one gather call)
NR = NP + NI                 # accumulator rows incl. trash window
EPS_T = 1e-3
EPS_U = 0.01
CW_COEF = float(np.sqrt(2.0) / np.sqrt(DT))

_PROG_CACHE = {}


def _remap(n):
    return (n % P) * NB + n // P


def _wrap16(idx):
    w = np.asarray(idx).reshape(NI // 16, 16).T.astype(np.int16)
    return np.tile(w, (8, 1))


def _group_edges(i, j, cap=NI):
    """Partition edge ids into groups of <=cap with all-distinct i and j.
    Each round takes the first-occurrence set for both endpoints (any subset
    of it is still distinct) and emits only full cap-sized groups, recycling
    the remainder so almost every group is full."""
    rem = np.arange(len(i))
    groups = []
    while len(rem):
        fi = np.zeros(len(rem), bool)
        fj = np.zeros(len(rem), bool)
        _, ui = np.unique(i[rem], return_index=True)
        _, uj = np.unique(j[rem], return_index=True)
        fi[ui] = True
        fj[uj] = True
        take = np.where(fi & fj)[0]
        nfull = len(take) // cap
        if nfull >= 1 and len(take) > nfull * cap and len(rem) > len(take):
            take = take[:nfull * cap]          # recycle the partial group
        for o in range(0, len(take), cap):
            groups.append(rem[take[o:o + cap]])
        keep = np.ones(len(rem), bool)
        keep[take] = False
        rem = rem[keep]
    return groups


def _build_program(nsc, b3):
    nc = bacc.Bacc("TRN2", target_bir_lowering=False, debug=False,
                   num_devices=NCORES)
    ETOT = nsc * ET

    t_in = lambda nm, shp, dt=F32: nc.dram_tensor(nm, shp, dt, kind="ExternalInput").ap()
    S_nm = t_in("S_nm", [P, NB])
    d_nm = t_in("d_nm", [P, NB])
    v_nm = t_in("v_nm", [P, NB, 3])
    vv_half = t_in("vv_half", [P, NB])
    epack = t_in("epack", [P, ETOT, 14])
    gi_idx = t_in("gi_idx", [nsc, P, 2 * (NI // 16)], I16)
    gj_idx = t_in("gj_idx", [nsc, P, 2 * (NI // 16)], I16)
    si_idx = t_in("si_idx", [nsc, P, 2 * (NI // 16)], I16)
    sj_idx = t_in("sj_idx", [nsc, P, 2 * (NI // 16)], I16)
    U1 = t_in("U1", [2, HID])
    U2 = t_in("U2", [P, P])
    U3 = t_in("U3", [P, 2])
    Ub1v = t_in("Ub1v", [HID, 4])
    Ub2 = t_in("Ub2", [P, 1])
    L1_ABi = t_in("L1_ABi", [4, P])
    L1_ABj = t_in("L1_ABj", [4, P])
    L1_CWi = t_in("L1_CWi", [4, P])
    L1_CWj = t_in("L1_CWj", [4, P])
    L2_AB = t_in("L2_AB", [P, P])
    L2_CW = t_in("L2_CW", [P, P])
    L3_AB = t_in("L3_AB", [P, 32])
    L3_CW = t_in("L3_CW", [P, 32])
    B1_AB = t_in("B1_AB", [P, 2])
    B1_CW = t_in("B1_CW", [P, 2])
    B2_AB = t_in("B2_AB", [P, 1])
    B2_CW = t_in("B2_CW", [P, 1])

    acc = nc.dram_tensor("acc", [NR, 64], F32, kind="Internal").ap()
    acc_out = nc.dram_tensor("acc_out", [NP, 8], F32, kind="ExternalOutput").ap()
    invT_out = nc.dram_tensor("invT_out", [P, NB], F32, kind="ExternalOutput").ap()
    eout = nc.dram_tensor("eout", [P, NB], F32, kind="ExternalOutput").ap()

    node_tab = nc.dram_tensor("node_tab", [NP * 64], F32, kind="Internal").ap()
    sv_stage = nc.dram_tensor("sv_stage", [2, NP], F32, kind="Internal").ap()
    u_stage = nc.dram_tensor("u_stage", [4, NP], F32, kind="Internal").ap()

    with tile.TileContext(nc) as tc:
        with tc.tile_pool(name="const", bufs=1) as cpool:
            ident = cpool.tile([P, P], F32)
            make_identity(nc, ident[:])

            def ldc(ap_in, shape, dt=F32):
                t = cpool.tile(shape, dt, tag=ap_in.tensor.name, name=ap_in.tensor.name)
                nc.sync.dma_start(out=t[:], in_=ap_in)
                return t

            cU1 = ldc(U1, [2, HID]); cU2 = ldc(U2, [P, P]); cU3 = ldc(U3, [P, 2])
            cUb1 = ldc(Ub1v, [HID, 4]); cUb2 = ldc(Ub2, [P, 1])
            cL1 = {"ABi": ldc(L1_ABi, [4, P]), "ABj": ldc(L1_ABj, [4, P]),
                   "CWi": ldc(L1_CWi, [4, P]), "CWj": ldc(L1_CWj, [4, P])}
            cL2 = {"AB": ldc(L2_AB, [P, P]), "CW": ldc(L2_CW, [P, P])}
            cL3 = {"AB": ldc(L3_AB, [P, 32]), "CW": ldc(L3_CW, [P, 32])}
            cB1 = {"AB": ldc(B1_AB, [P, 2]), "CW": ldc(B1_CW, [P, 2])}
            cB2 = {"AB": ldc(B2_AB, [P, 1]), "CW": ldc(B2_CW, [P, 1])}

            # ================= node phase =================
            with tc.tile_pool(name="node", bufs=1) as npool, \
                 tc.tile_pool(name="npsA", bufs=1, space="PSUM") as npsA, \
                 tc.tile_pool(name="npsB", bufs=1, space="PSUM") as npsB:
                nS = npool.tile([P, NB], F32)
                nD = npool.tile([P, NB], F32)
                nV = npool.tile([P, NB], F32)
                nc.sync.dma_start(out=nS[:], in_=S_nm[:])
                nc.sync.dma_start(out=nD[:], in_=d_nm[:])
                nc.vector.reciprocal(out=nV[:], in_=nD[:])
                nc.sync.dma_start(out=sv_stage[0, :].rearrange("(p b) -> p b", p=P), in_=nS[:])
                nc.sync.dma_start(out=sv_stage[1, :].rearrange("(p b) -> p b", p=P), in_=nV[:])

                CH = 2048
                off = 0
                while off < NP:
                    cw = min(CH, NP - off)
                    sv = npool.tile([2, CH], F32, tag="sv", name="sv")
                    nc.sync.dma_start(out=sv[:, :cw], in_=sv_stage[:, off:off + cw])
                    ps1 = npsA.tile([P, CH], F32, tag="ups", name="ups")
                    o = 0
                    while o < cw:
                        n = min(512, cw - o)
                        nc.tensor.matmul(out=ps1[:HID, o:o + n], lhsT=cU1[:],
                                         rhs=sv[:, o:o + n], start=True, stop=True)
                        o += n
                    h1a = npool.tile([P, CH], F32, tag="h1a", name="h1a")
                    h1b = npool.tile([P, CH], F32, tag="h1b", name="h1b")
                    for dst, bcol in ((h1a[:HID, :cw], 0), (h1a[HID:, :cw], 1),
                                      (h1b[:HID, :cw], 2), (h1b[HID:, :cw], 3)):
                        nc.scalar.activation(out=dst, in_=ps1[:HID, :cw], func=AF.Exp, bias=cUb1[:, bcol:bcol + 1])
                        nc.vector.tensor_scalar(out=dst, in0=dst, scalar1=1.0, scalar2=None, op0=TT.add)
                        nc.scalar.activation(out=dst, in_=dst, func=AF.Ln)
                    for hbuf, r0 in ((h1a, 0), (h1b, 2)):
                        ps2 = npsA.tile([P, CH], F32, tag="ups", name="ups")
                        o = 0
                        while o < cw:
                            n = min(512, cw - o)
                            nc.tensor.matmul(out=ps2[:, o:o + n], lhsT=cU2[:],
                                             rhs=hbuf[:, o:o + n], start=True, stop=True)
                            o += n
                        h2 = npool.tile([P, CH], F32, tag="uh2", name="uh2")
                        nc.scalar.activation(out=h2[:, :cw], in_=ps2[:, :cw], func=AF.Exp, bias=cUb2[:, 0:1])
                        nc.vector.tensor_scalar(out=h2[:, :cw], in0=h2[:, :cw], scalar1=1.0, scalar2=None, op0=TT.add)
                        nc.scalar.activation(out=h2[:, :cw], in_=h2[:, :cw], func=AF.Ln)
                        ps3 = npsB.tile([2, CH], F32, tag="ups3", name="ups3")
                        o = 0
                        while o < cw:
                            n = min(512, cw - o)
                            nc.tensor.matmul(out=ps3[:, o:o + n], lhsT=cU3[:],
                                             rhs=h2[:, o:o + n], start=True, stop=True)
                            o += n
                        uo = npool.tile([2, CH], F32, tag="uo", name="uo")
                        nc.scalar.copy(out=uo[:, :cw], in_=ps3[:, :cw])
                        nc.sync.dma_start(out=u_stage[r0:r0 + 2, off:off + cw], in_=uo[:, :cw])
                    off += cw

                uem = []
                for r in range(4):
                    t = npool.tile([P, NB], F32, tag=f"uem{r}", name=f"uem{r}")
                    nc.sync.dma_start(out=t[:], in_=u_stage[r, :].rearrange("(p b) -> p b", p=P))
                    uem.append(t)
                U0, USp, UVp, USm = uem
                tt = lambda tg: npool.tile([P, NB], F32, tag=tg, name=tg)
                T = tt("T"); Pm = tt("Pm"); den = tt("den")
                invT = tt("invT"); invC = tt("invC"); invCT = tt("invCT"); Pd2 = tt("Pd2")
                tmp = tt("ntmp"); tmp2 = tt("ntmp2")
                vt_ = nc.vector.tensor_tensor
                vt_(out=tmp[:], in0=USp[:], in1=U0[:], op=TT.subtract)
                nc.scalar.mul(out=T[:], in_=tmp[:], mul=1.0 / EPS_U)
                vt_(out=tmp[:], in0=U0[:], in1=UVp[:], op=TT.subtract)
                nc.scalar.mul(out=Pm[:], in_=tmp[:], mul=1.0 / EPS_U)
                vt_(out=tmp[:], in0=USp[:], in1=USm[:], op=TT.add)
                vt_(out=tmp2[:], in0=U0[:], in1=U0[:], op=TT.add)
                vt_(out=den[:], in0=tmp[:], in1=tmp2[:], op=TT.subtract)
                nc.vector.reciprocal(out=invT[:], in_=T[:])
                vt_(out=tmp[:], in0=den[:], in1=invT[:], op=TT.mult)
                nc.scalar.mul(out=invC[:], in_=tmp[:], mul=1.0 / (EPS_U * EPS_U))
                vt_(out=invCT[:], in0=invC[:], in1=invT[:], op=TT.mult)
                vt_(out=tmp[:], in0=nV[:], in1=nV[:], op=TT.mult)
                vt_(out=Pd2[:], in0=Pm[:], in1=tmp[:], op=TT.mult)
                nvv = npool.tile([P, NB], F32, tag="nvv", name="nvv")
                nc.sync.dma_start(out=nvv[:], in_=vv_half[:])
                eo = tt("eo")
                vt_(out=eo[:], in0=U0[:], in1=nvv[:], op=TT.add)
                nc.sync.dma_start(out=eout[:], in_=eo[:])
                nc.sync.dma_start(out=invT_out[:], in_=invT[:])

                ntab = npool.tile([P, NB * 64], F32, tag="ntab", name="ntab")
                nc.vector.memset(ntab[:], 0.0)
                ntv = ntab[:].rearrange("p (b c) -> p b c", c=64)
                for ci, src in enumerate((T, invT, invCT, invC, Pd2)):
                    nc.vector.tensor_copy(out=ntv[:, :, ci], in_=src[:])
                nvt = npool.tile([P, NB, 3], F32, tag="nvt", name="nvt")
                nc.sync.dma_start(out=nvt[:], in_=v_nm[:])
                for c3 in range(3):
                    nc.vector.tensor_copy(out=ntv[:, :, 5 + c3], in_=nvt[:, :, c3])
                nc.sync.dma_start(out=node_tab[:].rearrange("(p q) -> p q", p=P), in_=ntab[:])

            # ================= edge phase =================
            # acc is Internal (not PJRT zero-donated): zero it on device first
            with tc.tile_pool(name="zpool", bufs=1) as zpool:
                ztile = zpool.tile([P, 4096], F32)
                nc.vector.memset(ztile[:], 0.0)
                accf = acc[:, :].rearrange("r c -> (r c)").rearrange("(p q) -> p q", p=P)
                ACCQ = NR * 64 // P          # 10560 elems per partition
                o = 0
                while o < ACCQ:
                    n = min(4096, ACCQ - o)
                    nc.sync.dma_start(out=accf[:, o:o + n], in_=ztile[:, :n])
                    o += n
            ntab_rows = node_tab[:].rearrange("(r c) -> r c", c=64)
            with tc.tile_pool(name="sbuf", bufs=2) as pool, \
                 tc.tile_pool(name="mlp", bufs=1) as mpool, \
                 tc.tile_pool(name="pA", bufs=2, space="PSUM") as pA, \
                 tc.tile_pool(name="pS", bufs=2, space="PSUM") as pS, \
                 tc.tile_pool(name="pT", bufs=2, space="PSUM") as pT:
                vt = nc.vector.tensor_tensor
                tsc = nc.vector.tensor_scalar
                for sc in range(nsc):
                    ed = pool.tile([P, ET, 14], F32, tag="ed", name="ed")
                    nc.sync.dma_start(out=ed[:], in_=epack[:, sc * ET:(sc + 1) * ET, :])
                    gii = pool.tile([P, 2 * (NI // 16)], I16, tag="gii", name="gii")
                    gji = pool.tile([P, 2 * (NI // 16)], I16, tag="gji", name="gji")
                    sii = pool.tile([P, 2 * (NI // 16)], I16, tag="sii", name="sii")
                    sji = pool.tile([P, 2 * (NI // 16)], I16, tag="sji", name="sji")
                    for t, src in ((gii, gi_idx), (gji, gj_idx), (sii, si_idx), (sji, sj_idx)):
                        nc.sync.dma_start(out=t[:], in_=src[sc])
                    gi = pool.tile([P, ET, 64], F32, tag="gi", name="gi")
                    gj = pool.tile([P, ET, 64], F32, tag="gj", name="gj")
                    for gt, it in ((gi, gii), (gj, gji)):
                        for hf in range(2):
                            nc.gpsimd.dma_gather(
                                out_ap=gt[:, hf * (ET // 2):(hf + 1) * (ET // 2), :],
                                in_ap=ntab_rows,
                                idxs_ap=it[:, hf * (NI // 16):(hf + 1) * (NI // 16)],
                                num_idxs=NI, num_idxs_reg=NI, elem_size=64)

                    g = lambda tg: pool.tile([P, ET], F32, tag=tg, name=tg)
                    vij = [g(f"vij{c}") for c in range(3)]
                    for c in range(3):
                        vt(out=vij[c][:], in0=gi[:, :, 5 + c], in1=gj[:, :, 5 + c], op=TT.subtract)
                    r2 = g("r2"); tmpe = g("tmpe"); tmpf = g("tmpf")
                    vt(out=r2[:], in0=ed[:, :, 0], in1=ed[:, :, 0], op=TT.mult)
                    for c in (1, 2):
                        vt(out=tmpe[:], in0=ed[:, :, c], in1=ed[:, :, c], op=TT.mult)
                        vt(out=r2[:], in0=r2[:], in1=tmpe[:], op=TT.add)
                    rpl = g("rpl")
                    nc.scalar.activation(out=rpl[:], in_=r2[:], func=AF.Sqrt)
                    rinv = g("rinv")
                    tsc(out=rinv[:], in0=rpl[:], scalar1=1e-8, scalar2=None, op0=TT.add)
                    nc.vector.reciprocal(out=rinv[:], in_=rinv[:])
                    epl = [g(f"e{c}") for c in range(3)]
                    for c in range(3):
                        vt(out=epl[c][:], in0=ed[:, :, c], in1=rinv[:], op=TT.mult)
                    ev = g("ev"); vv = g("vv")
                    vt(out=ev[:], in0=epl[0][:], in1=vij[0][:], op=TT.mult)
                    vt(out=vv[:], in0=vij[0][:], in1=vij[0][:], op=TT.mult)
                    for c in (1, 2):
                        vt(out=tmpe[:], in0=epl[c][:], in1=vij[c][:], op=TT.mult)
                        vt(out=ev[:], in0=ev[:], in1=tmpe[:], op=TT.add)
                        vt(out=tmpe[:], in0=vij[c][:], in1=vij[c][:], op=TT.mult)
                        vt(out=vv[:], in0=vv[:], in1=tmpe[:], op=TT.add)

                    stg = pool.tile([P, ET, 4], F32, tag="stg", name="stg")
                    nc.vector.tensor_copy(out=stg[:, :, 0], in_=rpl[:])
                    nc.vector.tensor_copy(out=stg[:, :, 1], in_=gi[:, :, 0])
                    nc.vector.tensor_copy(out=stg[:, :, 2], in_=gj[:, :, 0])
                    nc.vector.memset(stg[:, :, 3], 0.0)
                    xt = mpool.tile([4, ET * P], F32, tag="xt", name="xt")
                    for t in range(ET):
                        pst = pT.tile([P, P], F32, tag="tp", name="tp")
                        nc.tensor.transpose(out=pst[:4, :], in_=stg[:, t, :], identity=ident[:])
                        nc.vector.tensor_copy(out=xt[:, t * P:(t + 1) * P], in_=pst[:4, :])

                    # l1: per half, 4 stationaries, silu x2 bias variants
                    h1 = {k: (mpool.tile([P, SC], F32, tag=f"h1{k}0", name=f"h1{k}0"),
                              mpool.tile([P, SC], F32, tag=f"h1{k}1", name=f"h1{k}1"))
                          for k in ("ABi", "ABj", "CWi", "CWj")}
                    for hf in range(2):
                        for key in ("ABi", "ABj", "CWi", "CWj"):
                            bt = cB1["AB" if key.startswith("AB") else "CW"]
                            psl = pA.tile([P, HSC], F32, tag="A", name="A")
                            for t8 in range(8):
                                t = hf * 8 + t8
                                nc.tensor.matmul(out=psl[:, t8 * P:(t8 + 1) * P],
                                                 lhsT=cL1[key][:],
                                                 rhs=xt[:, t * P:(t + 1) * P],
                                                 start=True, stop=True)
                            sl = slice(hf * HSC, (hf + 1) * HSC)
                            nc.scalar.activation(out=h1[key][0][:, sl], in_=psl[:], func=AF.Silu, bias=bt[:, 0:1])
                            nc.scalar.activation(out=h1[key][1][:, sl], in_=psl[:], func=AF.Silu, bias=bt[:, 1:2])

                    # l2 + l3 stacks (per side, per half)
                    sv_i = mpool.tile([P, SC], F32, tag="sv_i", name="sv_i")
                    sv_j = mpool.tile([P, SC], F32, tag="sv_j", name="sv_j")
                    for side, svt in (("i", sv_i), ("j", sv_j)):
                        for hf in range(2):
                            stks = [pS.tile([P, 512], F32, tag="S", name="S") for _ in range(2)]
                            for vi, (key, var) in enumerate(((f"AB{side}", 0), (f"AB{side}", 1),
                                                            (f"CW{side}", 0), (f"CW{side}", 1))):
                                nm2 = "AB" if key.startswith("AB") else "CW"
                                hin = h1[key][var]
                                ps2 = pA.tile([P, HSC], F32, tag="A", name="A")
                                for o in (0, 512):
                                    nc.tensor.matmul(out=ps2[:, o:o + 512], lhsT=cL2[nm2][:],
                                                     rhs=hin[:, hf * HSC + o:hf * HSC + o + 512],
                                                     start=True, stop=True)
                                h2 = mpool.tile([P, HSC], F32, tag="h2", name="h2")
                                nc.scalar.activation(out=h2[:], in_=ps2[:], func=AF.Silu, bias=cB2[nm2][:, 0:1])
                                for ci in range(2):
                                    nc.tensor.matmul(out=stks[ci][32 * vi:32 * (vi + 1), :],
                                                     lhsT=cL3[nm2][:], rhs=h2[:, ci * 512:(ci + 1) * 512],
                                                     start=True, stop=True,
                                                     tile_position=(0, 32 * vi))
                            for ci in range(2):
                                nc.vector.tensor_copy(out=svt[:, hf * HSC + ci * 512:hf * HSC + (ci + 1) * 512],
                                                      in_=stks[ci][:])

                    vem_i = pool.tile([P, ET, P], F32, tag="vem_i", name="vem_i")
                    vem_j = pool.tile([P, ET, P], F32, tag="vem_j", name="vem_j")
                    for svt, vem in ((sv_i, vem_i), (sv_j, vem_j)):
                        for t in range(ET):
                            pst = pT.tile([P, P], F32, tag="tp", name="tp")
                            nc.tensor.transpose(out=pst[:], in_=svt[:, t * P:(t + 1) * P], identity=ident[:])
                            nc.vector.tensor_copy(out=vem[:, t, :], in_=pst[:])

                    # ---- payload combine ----
                    pb = g
                    def mkval(dst, vem, v, r, bias):
                        tsc(out=dst[:], in0=vem[:, :, 32 * v + r], scalar1=float(bias), scalar2=None, op0=TT.add)
                    A_i = pb("A_i"); B_i = pb("B_i"); A_ie = pb("A_ie"); B_ie = pb("B_ie")
                    C_i = pb("C_i"); Wp = pb("Wp"); C_ie = pb("C_ie"); Wm = pb("Wm")
                    A_j = pb("A_j"); B_j = pb("B_j"); A_je = pb("A_je"); B_je = pb("B_je")
                    C_j = pb("C_j"); C_je = pb("C_je")
                    mkval(A_i, vem_i, 0, 0, b3['A']); mkval(B_i, vem_i, 0, 1, b3['B'])
                    mkval(A_ie, vem_i, 1, 0, b3['A']); mkval(B_ie, vem_i, 1, 1, b3['B'])
                    mkval(C_i, vem_i, 2, 0, b3['C']); mkval(C_ie, vem_i, 3, 0, b3['C'])
                    mkval(A_j, vem_j, 0, 0, b3['A']); mkval(B_j, vem_j, 0, 1, b3['B'])
                    mkval(A_je, vem_j, 1, 0, b3['A']); mkval(C_j, vem_j, 2, 0, b3['C'])
                    mkval(B_je, vem_j, 1, 1, b3['B']); mkval(C_je, vem_j, 3, 0, b3['C'])
                    for dst, v, sgn in ((Wp, 2, 1.0), (Wm, 3, -1.0)):
                        tsc(out=tmpe[:], in0=vem_i[:, :, 32 * v + 1], scalar1=float(b3['W']), scalar2=None, op0=TT.add)
                        nc.scalar.activation(out=tmpe[:], in_=tmpe[:], func=AF.Exp)
                        tsc(out=tmpf[:], in0=rpl[:], scalar1=sgn * EPS_T / H, scalar2=None, op0=TT.add)
                        vt(out=tmpf[:], in0=tmpf[:], in1=tmpf[:], op=TT.mult)
                        tsc(out=tmpf[:], in0=tmpf[:], scalar1=-1.0, scalar2=1.0, op0=TT.mult, op1=TT.add)
                        vt(out=dst[:], in0=tmpe[:], in1=tmpf[:], op=TT.mult)
                    dWdr = pb("dWdr")
                    vt(out=dWdr[:], in0=Wp[:], in1=Wm[:], op=TT.subtract)
                    tsc(out=dWdr[:], in0=dWdr[:], scalar1=1.0 / (2 * EPS_T), scalar2=None, op0=TT.mult)

                    A_ij = pb("A_ij"); B_ij = pb("B_ij"); C_ij = pb("C_ij")
                    vt(out=A_ij[:], in0=A_i[:], in1=A_j[:], op=TT.mult)
                    vt(out=B_ij[:], in0=B_i[:], in1=B_j[:], op=TT.mult)
                    vt(out=C_ij[:], in0=C_i[:], in1=C_j[:], op=TT.mult)

                    def grad(dst, Pij, Xe, Xo):
                        vt(out=tmpe[:], in0=Xe[:], in1=Xo[:], op=TT.mult)
                        vt(out=tmpe[:], in0=tmpe[:], in1=Pij[:], op=TT.subtract)
                        vt(out=tmpe[:], in0=tmpe[:], in1=Pij[:], op=TT.mult)
                        tsc(out=dst[:], in0=tmpe[:], scalar1=2.0 / EPS_T, scalar2=None, op0=TT.mult)
                    gA_i = pb("gA_i"); gB_i = pb("gB_i"); gC_i = pb("gC_i")
                    gA_j = pb("gA_j"); gB_j = pb("gB_j"); gC_j = pb("gC_j")
                    grad(gA_i, A_ij, A_ie, A_j); grad(gB_i, B_ij, B_ie, B_j)
                    grad(gC_i, C_ij, C_ie, C_j)
                    grad(gA_j, A_ij, A_je, A_i); grad(gB_j, B_ij, B_je, B_i)
                    grad(gC_j, C_ij, C_je, C_i)

                    Ti = gi[:, :, 0]; invTi = gi[:, :, 1]; invCiTi = gi[:, :, 2]
                    invCi = gi[:, :, 3]; Pd2i = gi[:, :, 4]
                    Tj = gj[:, :, 0]; invTj = gj[:, :, 1]; invCjTj = gj[:, :, 2]
                    invCj = gj[:, :, 3]; Pd2j = gj[:, :, 4]

                    sTin = pb("sTin"); sCTin = pb("sCTin")
                    vt(out=sTin[:], in0=invTi, in1=invTj, op=TT.add)
                    vt(out=sCTin[:], in0=invCiTi, in1=invCjTj, op=TT.add)
                    a2h = pb("a2h"); coef = pb("coef")
                    vt(out=tmpe[:], in0=A_ij[:], in1=A_ij[:], op=TT.mult)
                    tsc(out=a2h[:], in0=tmpe[:], scalar1=0.5, scalar2=None, op0=TT.mult)
                    vt(out=tmpf[:], in0=B_ij[:], in1=B_ij[:], op=TT.mult)
                    vt(out=tmpf[:], in0=tmpf[:], in1=tmpe[:], op=TT.subtract)
                    tsc(out=tmpf[:], in0=tmpf[:], scalar1=1.0 / D, scalar2=None, op0=TT.mult)
                    vt(out=coef[:], in0=a2h[:], in1=tmpf[:], op=TT.add)
                    term6 = pb("term6")
                    tsc(out=term6[:], in0=a2h[:], scalar1=float(D), scalar2=None, op0=TT.mult)
                    vt(out=term6[:], in0=term6[:], in1=coef[:], op=TT.add)
                    tsc(out=term6[:], in0=term6[:], scalar1=-1.0, scalar2=None, op0=TT.mult)

                    def gcoef(dst, gA, gB):
                        vt(out=tmpe[:], in0=gB[:], in1=gA[:], op=TT.subtract)
                        tsc(out=tmpe[:], in0=tmpe[:], scalar1=1.0 / D, scalar2=None, op0=TT.mult)
                        tsc(out=tmpf[:], in0=gA[:], scalar1=0.5, scalar2=None, op0=TT.mult)
                        vt(out=dst[:], in0=tmpf[:], in1=tmpe[:], op=TT.add)
                    gco_i = pb("gco_i"); gco_j = pb("gco_j")
                    gcoef(gco_i, gA_i, gB_i); gcoef(gco_j, gA_j, gB_j)
                    gAh_i = pb("gAh_i"); gAh_j = pb("gAh_j")
                    tsc(out=gAh_i[:], in0=gA_i[:], scalar1=0.5, scalar2=None, op0=TT.mult)
                    tsc(out=gAh_j[:], in0=gA_j[:], scalar1=0.5, scalar2=None, op0=TT.mult)

                    tr = pb("tr"); trD = pb("trD")
                    vt(out=tr[:], in0=ed[:, :, 3], in1=ed[:, :, 7], op=TT.add)
                    vt(out=tr[:], in0=tr[:], in1=ed[:, :, 11], op=TT.add)
                    tsc(out=trD[:], in0=tr[:], scalar1=1.0 / D, scalar2=None, op0=TT.mult)
                    termw = [pb(f"tw{c}") for c in range(3)]
                    for a in range(3):
                        for bb in range(3):
                            vt(out=tmpe[:], in0=ed[:, :, 3 + 3 * a + bb], in1=ed[:, :, 3 + 3 * bb + a], op=TT.add)
                            tsc(out=tmpe[:], in0=tmpe[:], scalar1=0.5, scalar2=None, op0=TT.mult)
                            if a == bb:
                                vt(out=tmpe[:], in0=tmpe[:], in1=trD[:], op=TT.subtract)
                            vt(out=tmpe[:], in0=tmpe[:], in1=A_ij[:], op=TT.mult)
                            if a == bb:
                                vt(out=tmpf[:], in0=B_ij[:], in1=trD[:], op=TT.mult)
                                vt(out=tmpe[:], in0=tmpe[:], in1=tmpf[:], op=TT.add)
                            vt(out=tmpe[:], in0=tmpe[:], in1=epl[bb][:], op=TT.mult)
                            if bb == 0:
                                nc.vector.tensor_copy(out=termw[a][:], in_=tmpe[:])
                            else:
                                vt(out=termw[a][:], in0=termw[a][:], in1=tmpe[:], op=TT.add)
                    termSw = pb("termSw")
                    vt(out=termSw[:], in0=termw[0][:], in1=vij[0][:], op=TT.mult)
                    for c in (1, 2):
                        vt(out=tmpe[:], in0=termw[c][:], in1=vij[c][:], op=TT.mult)
                        vt(out=termSw[:], in0=termSw[:], in1=tmpe[:], op=TT.add)
                    tsc(out=termSw[:], in0=termSw[:], scalar1=-0.5, scalar2=None, op0=TT.mult)

                    sPd = pb("sPd")
                    vt(out=sPd[:], in0=Pd2i, in1=Pd2j, op=TT.add)
                    vt(out=sPd[:], in0=sPd[:], in1=dWdr[:], op=TT.mult)
                    stt = pb("stt")
                    vt(out=stt[:], in0=sTin[:], in1=sCTin[:], op=TT.subtract)
                    cvij = pb("cvij")
                    vt(out=cvij[:], in0=stt[:], in1=a2h[:], op=TT.mult)
                    vt(out=tmpe[:], in0=gAh_i[:], in1=invCi, op=TT.mult)
                    vt(out=cvij[:], in0=cvij[:], in1=tmpe[:], op=TT.add)
                    vt(out=tmpe[:], in0=gAh_j[:], in1=invCj, op=TT.mult)
                    vt(out=cvij[:], in0=cvij[:], in1=tmpe[:], op=TT.add)
                    ce = pb("ce")
                    vt(out=ce[:], in0=stt[:], in1=coef[:], op=TT.mult)
                    vt(out=tmpe[:], in0=gco_i[:], in1=invCi, op=TT.mult)
                    vt(out=ce[:], in0=ce[:], in1=tmpe[:], op=TT.add)
                    vt(out=tmpe[:], in0=gco_j[:], in1=invCj, op=TT.mult)
                    vt(out=ce[:], in0=ce[:], in1=tmpe[:], op=TT.add)
                    vt(out=ce[:], in0=ce[:], in1=ev[:], op=TT.mult)
                    wm = ed[:, :, 13]
                    u3 = [pb(f"u3{c}") for c in range(3)]
                    for c in range(3):
                        vt(out=tmpe[:], in0=cvij[:], in1=vij[c][:], op=TT.mult)
                        vt(out=tmpf[:], in0=ce[:], in1=epl[c][:], op=TT.mult)
                        vt(out=tmpe[:], in0=tmpe[:], in1=tmpf[:], op=TT.add)
                        tsc(out=tmpe[:], in0=tmpe[:], scalar1=-0.5, scalar2=None, op0=TT.mult)
                        vt(out=tmpf[:], in0=sPd[:], in1=epl[c][:], op=TT.mult)
                        vt(out=tmpe[:], in0=tmpe[:], in1=tmpf[:], op=TT.subtract)
                        tsc(out=tmpf[:], in0=termw[c][:], scalar1=CW_COEF, scalar2=None, op0=TT.mult)
                        vt(out=tmpe[:], in0=tmpe[:], in1=tmpf[:], op=TT.add)
                        vt(out=u3[c][:], in0=tmpe[:], in1=wm, op=TT.mult)

                    aux2 = pb("aux2"); ev2 = pb("ev2")
                    vt(out=ev2[:], in0=ev[:], in1=ev[:], op=TT.mult)
                    vt(out=aux2[:], in0=a2h[:], in1=vv[:], op=TT.mult)
                    vt(out=tmpf[:], in0=coef[:], in1=ev2[:], op=TT.mult)
                    vt(out=aux2[:], in0=aux2[:], in1=tmpf[:], op=TT.add)
                    tsc(out=aux2[:], in0=aux2[:], scalar1=0.25, scalar2=None, op0=TT.mult)
                    C2 = pb("C2")
                    vt(out=C2[:], in0=C_ij[:], in1=C_ij[:], op=TT.mult)
                    dTin = pb("dTin")
                    vt(out=dTin[:], in0=invTi, in1=invTj, op=TT.subtract)
                    sm1 = pb("sm1"); sm2 = pb("sm2")
                    vt(out=tmpe[:], in0=sTin[:], in1=aux2[:], op=TT.mult)
                    vt(out=tmpf[:], in0=dTin[:], in1=C2[:], op=TT.mult)
                    vt(out=sm1[:], in0=tmpe[:], in1=tmpf[:], op=TT.add)
                    vt(out=sm2[:], in0=tmpe[:], in1=tmpf[:], op=TT.subtract)
                    two_i = pb("two_i"); t1S = pb("t1S"); t4 = pb("t4")
                    tsc(out=two_i[:], in0=invCiTi, scalar1=2.0, scalar2=None, op0=TT.mult)
                    vt(out=tmpe[:], in0=two_i[:], in1=invCjTj, op=TT.add)
                    vt(out=t1S[:], in0=tmpe[:], in1=aux2[:], op=TT.mult)
                    tsc(out=t1S[:], in0=t1S[:], scalar1=-1.0, scalar2=None, op0=TT.mult)
                    vt(out=tmpe[:], in0=two_i[:], in1=invCjTj, op=TT.subtract)
                    vt(out=t4[:], in0=tmpe[:], in1=C2[:], op=TT.mult)
                    tsc(out=t4[:], in0=t4[:], scalar1=-1.0, scalar2=None, op0=TT.mult)
                    t2S = pb("t2S")
                    vt(out=tmpe[:], in0=gAh_i[:], in1=vv[:], op=TT.mult)
                    vt(out=tmpf[:], in0=gco_i[:], in1=ev2[:], op=TT.mult)
                    vt(out=tmpe[:], in0=tmpe[:], in1=tmpf[:], op=TT.add)
                    vt(out=t2S[:], in0=tmpe[:], in1=invCi, op=TT.mult)
                    vt(out=tmpe[:], in0=gAh_j[:], in1=vv[:], op=TT.mult)
                    vt(out=tmpf[:], in0=gco_j[:], in1=ev2[:], op=TT.mult)
                    vt(out=tmpe[:], in0=tmpe[:], in1=tmpf[:], op=TT.add)
                    vt(out=tmpe[:], in0=tmpe[:], in1=invCj, op=TT.mult)
                    vt(out=t2S[:], in0=t2S[:], in1=tmpe[:], op=TT.add)
                    tsc(out=t2S[:], in0=t2S[:], scalar1=0.25, scalar2=None, op0=TT.mult)
                    t5 = pb("t5")
                    vt(out=t5[:], in0=gC_i[:], in1=invCi, op=TT.mult)
                    vt(out=tmpe[:], in0=gC_j[:], in1=invCj, op=TT.mult)
                    vt(out=t5[:], in0=t5[:], in1=tmpe[:], op=TT.subtract)
                    com = pb("com")
                    vt(out=com[:], in0=t1S[:], in1=t2S[:], op=TT.add)
                    vt(out=com[:], in0=com[:], in1=term6[:], op=TT.add)
                    cdv = pb("cdv")
                    vt(out=cdv[:], in0=C_ij[:], in1=ed[:, :, 12], op=TT.mult)
                    si = pb("si"); sj = pb("sj")
                    vt(out=si[:], in0=sm1[:], in1=com[:], op=TT.add)
                    vt(out=si[:], in0=si[:], in1=t4[:], op=TT.add)
                    vt(out=si[:], in0=si[:], in1=t5[:], op=TT.add)
                    vt(out=tmpe[:], in0=termSw[:], in1=cdv[:], op=TT.add)
                    tsc(out=tmpe[:], in0=tmpe[:], scalar1=CW_COEF, scalar2=None, op0=TT.mult)
                    vt(out=si[:], in0=si[:], in1=tmpe[:], op=TT.add)
                    vt(out=si[:], in0=si[:], in1=wm, op=TT.mult)
                    vt(out=sj[:], in0=sm2[:], in1=com[:], op=TT.add)
                    vt(out=sj[:], in0=sj[:], in1=t4[:], op=TT.subtract)
                    vt(out=sj[:], in0=sj[:], in1=t5[:], op=TT.subtract)
                    vt(out=tmpe[:], in0=termSw[:], in1=cdv[:], op=TT.subtract)
                    tsc(out=tmpe[:], in0=tmpe[:], scalar1=CW_COEF, scalar2=None, op0=TT.mult)
                    vt(out=sj[:], in0=sj[:], in1=tmpe[:], op=TT.add)
                    vt(out=sj[:], in0=sj[:], in1=wm, op=TT.mult)

                    qi = pool.tile([P, ET, 8], F32, tag="qi", name="qi")
                    qj = pool.tile([P, ET, 8], F32, tag="qj", name="qj")
                    nc.vector.memset(qi[:], 0.0)
                    nc.vector.memset(qj[:], 0.0)
                    for c in range(3):
                        nc.vector.tensor_copy(out=qi[:, :, c], in_=u3[c][:])
                        tsc(out=qj[:, :, c], in0=u3[c][:], scalar1=-1.0, scalar2=None, op0=TT.mult)
                    nc.vector.tensor_copy(out=qi[:, :, 3], in_=si[:])
                    nc.vector.tensor_copy(out=qj[:, :, 3], in_=sj[:])
                    for qt, it in ((qi, sii), (qj, sji)):
                        for hf in range(2):
                            nc.gpsimd.dma_scatter_add(
                                out_ap=acc[:, :8],
                                in_ap=qt[:, hf * (ET // 2):(hf + 1) * (ET // 2), :],
                                idxs_ap=it[:, hf * (NI // 16):(hf + 1) * (NI // 16)],
                                num_idxs=NI, num_idxs_reg=NI, elem_size=8, elem_step=64)

            with tc.tile_pool(name="fin", bufs=1) as fpool:
                accs = fpool.tile([P, NB, 8], F32)
                nc.sync.dma_start(out=accs[:], in_=acc[:NP, :8].rearrange("(p b) c -> p b c", p=P))
                nc.sync.dma_start(out=acc_out[:, :].rearrange("(p b) c -> p b c", p=P), in_=accs[:])

    nc.compile()
    return nc


def _prep_host(v, edge_index, r_ij, S, d, dW, dV, params):
    f32 = lambda x: np.asarray(x, np.float32)
    pr = {}
    for nm in ('W', 'A', 'B', 'C'):
        pr[nm] = [(f32(w), f32(b)) for (w, b) in params[nm]]
    pr['U'] = [(np.abs(f32(w)), f32(b)) for (w, b) in params['U']]

    i_all = np.asarray(edge_index[0], np.int64)
    j_all = np.asarray(edge_index[1], np.int64)

    cores = []
    slots_per_core = 0
    for c in range(NCORES):
        sl = slice(c * EPC, (c + 1) * EPC)
        groups = _group_edges(i_all[sl], j_all[sl])
        tok = sum((len(g) + NI - 1) // NI * NI for g in groups)
        slots_per_core = max(slots_per_core, tok)
        cores.append((sl, groups))
    nsc = (slots_per_core + SC - 1) // SC
    slots = nsc * SC

    pidx, bidx = np.meshgrid(np.arange(P), np.arange(NB), indexing='ij')
    nn = 128 * bidx + pidx
    valid = nn < N
    nnc = np.where(valid, nn, 0)
    S_nm = np.where(valid, f32(S)[nnc, 0], 0.0).astype(np.float32)
    d_nm = np.where(valid, f32(d)[nnc, 0], 1.0).astype(np.float32)
    v_npv = f32(v)
    v_nm = np.where(valid[..., None], v_npv[nnc], 0.0).astype(np.float32)
    vv_half = 0.5 * (v_nm ** 2).sum(-1).astype(np.float32)

    r_np = f32(r_ij)
    dW_np = f32(dW).reshape(E, 9)
    dV_np = f32(dV)

    ins = []
    for c in range(NCORES):
        sl, groups = cores[c]
        base = sl.start
        order = np.full(slots, -1, np.int64)
        pos = 0
        for gids in groups:
            order[pos:pos + len(gids)] = base + gids
            pos += (len(gids) + NI - 1) // NI * NI
        pad = order < 0
        oc = np.where(pad, 0, order)

        ep = np.zeros((slots, 14), np.float32)
        ep[:, 0:3] = r_np[oc]
        ep[:, 3:12] = dW_np[oc]
        ep[:, 12] = dV_np[oc, 0]
        ep[:, 13] = 1.0
        ep[pad, :] = 0.0
        ep[pad, 0] = 1.0
        epack = ep.reshape(slots // P, P, 14).transpose(1, 0, 2).copy()

        gi_t = np.where(pad, 0, _remap(i_all[oc]))
        gj_t = np.where(pad, 0, _remap(j_all[oc]))
        trash = NP + (np.arange(slots) % NI)
        si_t = np.where(pad, trash, _remap(i_all[oc]))
        sj_t = np.where(pad, trash, _remap(j_all[oc]))

        def wrap_calls(tokens):
            # token (s, h, c, p) -> out[s, p%16-replicated, h*64 + c]
            t = np.asarray(tokens, np.int16).reshape(nsc, 2, NI // 16, 16)
            w = t.transpose(0, 3, 1, 2).reshape(nsc, 16, 2 * (NI // 16))
            return np.tile(w, (1, 8, 1))

        ins.append({
            "S_nm": S_nm, "d_nm": d_nm, "v_nm": v_nm, "vv_half": vv_half,
            "epack": epack,
            "gi_idx": wrap_calls(gi_t), "gj_idx": wrap_calls(gj_t),
            "si_idx": wrap_calls(si_t), "sj_idx": wrap_calls(sj_t),
        })

    W1 = {nm: pr[nm][0] for nm in 'ABCW'}
    W2 = {nm: pr[nm][1] for nm in 'ABCW'}
    W3 = {nm: pr[nm][2] for nm in 'ABCW'}

    def l1_stat(side):
        ab = np.zeros((4, P), np.float32)
        cw = np.zeros((4, P), np.float32)
        trow = 1 if side == 'i' else 2
        for h, nm in ((0, 'A'), (1, 'B')):
            w = W1[nm][0]
            ab[0, h * 64:(h + 1) * 64] = w[0]
            ab[trow, h * 64:(h + 1) * 64] = w[1]
        cw[0, 0:64] = W1['C'][0][0]
        cw[trow, 0:64] = W1['C'][0][1]
        cw[0, 64:128] = W1['W'][0][0] / H
        return ab, cw

    L1_ABi_, L1_CWi_ = l1_stat('i')
    L1_ABj_, L1_CWj_ = l1_stat('j')
    z64 = np.zeros((64, 64), np.float32)
    blk = lambda a, b: np.block([[a, z64], [z64, b]]).astype(np.float32)
    L2_AB_ = blk(W2['A'][0], W2['B'][0])
    L2_CW_ = blk(W2['C'][0], W2['W'][0])
    L3_AB_ = np.zeros((P, 32), np.float32)
    L3_AB_[0:64, 0] = W3['A'][0][:, 0]; L3_AB_[64:128, 1] = W3['B'][0][:, 0]
    L3_CW_ = np.zeros((P, 32), np.float32)
    L3_CW_[0:64, 0] = W3['C'][0][:, 0]; L3_CW_[64:128, 1] = W3['W'][0][:, 0]

    B1_AB_ = np.zeros((P, 2), np.float32)
    B1_AB_[0:64, 0] = W1['A'][1]; B1_AB_[64:128, 0] = W1['B'][1]
    B1_AB_[0:64, 1] = W1['A'][1] + EPS_T * W1['A'][0][1]
    B1_AB_[64:128, 1] = W1['B'][1] + EPS_T * W1['B'][0][1]
    B1_CW_ = np.zeros((P, 2), np.float32)
    B1_CW_[0:64, 0] = W1['C'][1]
    B1_CW_[64:128, 0] = W1['W'][1] + (EPS_T / H) * W1['W'][0][0]
    B1_CW_[0:64, 1] = W1['C'][1] + EPS_T * W1['C'][0][1]
    B1_CW_[64:128, 1] = W1['W'][1] - (EPS_T / H) * W1['W'][0][0]
    B2_AB_ = np.concatenate([W2['A'][1], W2['B'][1]]).reshape(P, 1).astype(np.float32)
    B2_CW_ = np.concatenate([W2['C'][1], W2['W'][1]]).reshape(P, 1).astype(np.float32)
    b3 = {nm: float(W3[nm][1][0]) for nm in 'ABCW'}

    u1 = pr['U'][0][0]
    U1_ = np.stack([u1[0], -u1[1]]).astype(np.float32)
    U2_ = blk(pr['U'][1][0], pr['U'][1][0])
    U3_ = np.zeros((P, 2), np.float32)
    U3_[0:64, 0] = pr['U'][2][0][:, 0]; U3_[64:128, 1] = pr['U'][2][0][:, 0]
    b0 = pr['U'][0][1]
    Ub1v_ = np.stack([b0, b0 + EPS_U * u1[0], b0 - EPS_U * u1[1], b0 - EPS_U * u1[0]], 1).astype(np.float32)
    Ub2_ = np.concatenate([pr['U'][1][1], pr['U'][1][1]]).reshape(P, 1).astype(np.float32)
    b2U = float(pr['U'][2][1][0])

    wts = {
        "U1": U1_, "U2": U2_, "U3": U3_, "Ub1v": Ub1v_, "Ub2": Ub2_,
        "L1_ABi": L1_ABi_, "L1_ABj": L1_ABj_, "L1_CWi": L1_CWi_, "L1_CWj": L1_CWj_,
        "L2_AB": L2_AB_, "L2_CW": L2_CW_, "L3_AB": L3_AB_, "L3_CW": L3_CW_,
        "B1_AB": B1_AB_, "B1_CW": B1_CW_, "B2_AB": B2_AB_, "B2_CW": B2_CW_,
    }
    for m in ins:
        m.update(wts)
    return ins, nsc, b3, b2U


def _make_runner(nc):
    """Build the PJRT multi-core executable once (run_bass_via_pjrt re-traces
    and re-jits on every call; this caches the sharded callable)."""
    import jax
    import concourse.mybir as mb
    from concourse import bass2jax
    bass2jax.install_neuronx_cc_hook()
    in_names, out_names, out_avals = [], [], []
    pname = nc.partition_id_tensor.name if nc.partition_id_tensor else None
    for alloc in nc.m.functions[0].allocations:
        if not isinstance(alloc, mb.MemoryLocationSet):
            continue
        name = alloc.memorylocations[0].name
        if alloc.kind == "ExternalInput":
            if name != pname:
                in_names.append(name)
        elif alloc.kind == "ExternalOutput":
            out_names.append(name)
            out_avals.append(jax.core.ShapedArray(tuple(alloc.tensor_shape),
                                                  mb.dt.np(alloc.dtype)))
    n_params = len(in_names)
    all_in = in_names + out_names + ([pname] if pname else [])
    donate = tuple(range(n_params, n_params + len(out_names)))

    def _body(*args):
        operands = list(args)
        if pname is not None:
            operands.append(bass2jax.partition_id_tensor())
        return tuple(bass2jax._bass_exec_p.bind(
            *operands, out_avals=tuple(out_avals), in_names=tuple(all_in),
            out_names=tuple(out_names), lowering_input_output_aliases=(),
            sim_require_finite=True, sim_require_nnan=True, nc=nc))

    devices = jax.devices()[:NCORES]
    mesh = bass2jax.Mesh(np.asarray(devices), ("core",))
    specs = (bass2jax.PartitionSpec("core"),) * (n_params + len(out_names))
    sharded = jax.jit(bass2jax.shard_map(_body, mesh=mesh, in_specs=specs,
                                         out_specs=(bass2jax.PartitionSpec("core"),) * len(out_names),
                                         check_rep=False),
                      donate_argnums=donate, keep_unused=True)

    def run(in_maps):
        concat = [np.concatenate([np.asarray(m[nm]) for m in in_maps], axis=0)
                  for nm in in_names]
        zeros = [np.zeros((NCORES * a.shape[0], *a.shape[1:]), a.dtype)
                 for a in out_avals]
        outs = sharded(*concat, *zeros)
        return [{nm: np.asarray(outs[k]).reshape(NCORES, *out_avals[k].shape)[c]
                 for k, nm in enumerate(out_names)} for c in range(NCORES)]

    return run


def kernel(v, edge_index, r_ij, S, d, dW, dV, params):
    ins, nsc, b3, b2U = _prep_host(v, edge_index, r_ij, S, d, dW, dV, params)
    key = (nsc, tuple(sorted(b3.items())))
    if key not in _PROG_CACHE:
        nc = _build_program(nsc, b3)
        _PROG_CACHE[key] = (nc, _make_runner(nc))
    nc, runner = _PROG_CACHE[key]
    results = runner(ins)

    class _Res:
        pass
    res = _Res()
    res.results = results

    acc = np.zeros((NP, 8), np.float64)
    for c in range(NCORES):
        acc += res.results[c]["acc_out"]
    acc = acc.astype(np.float32)
    rows = _remap(np.arange(N))
    invT = res.results[0]["invT_out"]
    eoutv = res.results[0]["eout"]
    n_p = np.arange(N) % P
    n_b = np.arange(N) // P
    out0 = acc[rows, 0:3].astype(np.float32)
    out1 = (acc[rows, 3] * invT[n_p, n_b])[:, None].astype(np.float32)
    out2 = (eoutv[n_p, n_b][:, None] + b2U).astype(np.float32)
    return out0, out1, out2
